# revision 15
# baseline (speedup 1.0000x reference)
"""GAT 2-layer kernel for Trainium2, 8 NeuronCores (SPMD, dst-sharded).

Strategy (v4):
  - Destination-node sharding: core c owns nodes [c*6250,(c+1)*6250); edges bucketed
    into per-128-dst-node "slots" (49/core), padded to 128-edge chunks.
  - Stage A (replicated, batched x4): per 128-node tile one matmul computes
    [x@W1 (192) | alpha_src (3) | alpha_dst (3)]; xw+as go to a bf16 gather table
    G1 (512B rows); ad to slim f32 table AD1. Loads/stores batched 4 tiles per DMA
    to cut sequencer descriptor-generation. G1 split into two <=32768-row tensors
    (dma_gather int16 index limit).
  - Edge phase per layer, one SLOT (<=17 chunks) per iteration: two dma_gathers
    (one per table half) pull all source rows of the slot into one grow tile;
    indices/drel preloaded to SBUF once (no per-op JIT index DMAs -> no gather
    stalls). One-hot S (DVE is_equal vs iota, bf16) segment-reduces exp-weighted
    features AND softmax denominators per-slot in PSUM. alpha_dst[dst] expanded
    edge-wise via host-precomputed u8 drel-broadcast (DRELB) + DVE is_equal
    (replaces the PE ones-matmul broadcast) -> per-chunk S_T matmul vs the slot's
    alpha_dst block (preloaded: one batched indirect gather for L1; SBUF-resident
    for L2 -- no AD2 AllGather).
  - Per-slot epilogue: h = relu(sum/(denom+eps) + bias1); PE-transpose h, emit G2
    rows [h@W2 (64) bf16 | as2 f32] to DRAM and ad2 to SBUF; AllGather G2 only;
    layer 2 repeats the edge phase (1 head) against G2F views.
"""
import sys

sys.path.insert(0, "/opt/trn_rl_repo")
import numpy as np
import ml_dtypes

N = 50000
D = 128
HID = 64
H = 3
F1 = 192
F2 = 64
NCORES = 8
NPC = N // NCORES          # 6250 nodes per core
P = 128
NBLK = (NPC + P - 1) // P  # 49 slots per core
NT = (N + P - 1) // P      # 391 stage-A node tiles
NROW1 = NT * P             # 50048 G1 rows
HALF = 32768               # dma_gather int16 index limit
G1W = 256                  # bf16 cols: xw(192) | as f32 x3 (bf16 192:198) | pad
G2W = 128                  # bf16 cols: xw2(64) | as2 f32 (bf16 64:66) | pad
NROWC = NBLK * P           # 6272 rows per core shard
SLOPE = 0.2
EPS = 1e-16
BA = 4                     # stage-A tiles per DMA batch

_compiled = {}


def _host_prep(inputs):
    x = np.asarray(inputs["x"], dtype=np.float32)
    ei = np.asarray(inputs["edge_index"])
    W1 = np.asarray(inputs["W1"], dtype=np.float32)
    as1 = np.asarray(inputs["att_src1"], dtype=np.float32)
    ad1 = np.asarray(inputs["att_dst1"], dtype=np.float32)
    b1 = np.asarray(inputs["bias1"], dtype=np.float32)
    W2 = np.asarray(inputs["W2"], dtype=np.float32)
    as2 = np.asarray(inputs["att_src2"], dtype=np.float32)
    ad2 = np.asarray(inputs["att_dst2"], dtype=np.float32)
    b2 = np.asarray(inputs["bias2"], dtype=np.float32)

    loops = np.arange(N, dtype=np.int64)
    src = np.concatenate([ei[0].astype(np.int64), loops])
    dst = np.concatenate([ei[1].astype(np.int64), loops])
    order = np.argsort(dst, kind="stable")
    src = src[order]
    dst = dst[order]
    g2row = (src // NPC) * NROWC + (src % NPC)

    def build_layer(skey):
        # per-slot chunk counts (max over cores so program is uniform)
        core = dst // NPC
        rel = dst % NPC
        slot = rel // P
        half = (skey >= HALF).astype(np.int64)
        counts = np.zeros((NCORES, NBLK, 2), dtype=np.int64)
        np.add.at(counts, (core, slot, half), 1)
        Ka = np.ceil(counts[:, :, 0] / P).astype(np.int64).max(axis=0)
        Kb = np.ceil(counts[:, :, 1] / P).astype(np.int64).max(axis=0)
        Ktot = Ka + Kb
        NCH = int(Ktot.sum())
        GW = int(Ktot.max())
        # meta per slot: (chunk_base, Kb, Ka); b-chunks first within a slot
        meta = []
        cb = 0
        for s in range(NBLK):
            meta.append((cb, int(Kb[s]), int(Ka[s])))
            cb += int(Kb[s] + Ka[s])

        EPAD = NCH * P
        SRCK = np.zeros((NCORES, EPAD), dtype=np.int64)
        DREL = np.full((NCORES, EPAD), 255.0, dtype=np.float32)
        for c in range(NCORES):
            base_node = c * NPC
            cb = 0
            for s in range(NBLK):
                blo = base_node + s * P
                bhi = min(blo + P, base_node + NPC)
                lo = np.searchsorted(dst, blo, side="left")
                hi = np.searchsorted(dst, bhi, side="left")
                sk = skey[lo:hi]
                dr = (dst[lo:hi] - blo).astype(np.float32)
                a_mask = sk < HALF
                for which, KK, pad in ((~a_mask, Kb[s], HALF),
                                       (a_mask, Ka[s], 0)):
                    cnt = int(which.sum())
                    pos = cb * P
                    SRCK[c, pos:pos + cnt] = sk[which]
                    SRCK[c, pos + cnt:(cb + int(KK)) * P] = pad
                    DREL[c, pos:pos + cnt] = dr[which]
                    cb += int(KK)
        # device arrays
        DREL_t = np.ascontiguousarray(
            DREL.reshape(NCORES, NCH, P).transpose(0, 2, 1)
        ).astype(ml_dtypes.bfloat16)                       # [C, P, NCH]
        DRELB = np.ascontiguousarray(np.broadcast_to(
            DREL.astype(np.uint8).reshape(NCORES, 1, EPAD),
            (NCORES, P, EPAD)).transpose(1, 0, 2)).transpose(1, 0, 2)
        # ^ [C, P, NCH*P] u8, rows identical per partition
        # wrapped int16 indices, per chunk 8 cols: [P, NCH*8]
        IDXW = np.zeros((NCORES, P, NCH * 8), dtype=np.int16)
        for c in range(NCORES):
            for s in range(NBLK):
                c0, kb, ka = meta[s]
                for part, nch_p, base in ((0, kb, HALF), (kb, ka, 0)):
                    if nch_p == 0:
                        continue
                    lo = (c0 + part) * P
                    iv = SRCK[c, lo:lo + nch_p * P] - base
                    w = iv.reshape(-1, 16).T.astype(np.int16)  # [16, n/16]
                    IDXW[c, :, (c0 + part) * 8:(c0 + part + nch_p) * 8] = \
                        np.tile(w, (8, 1))
        return dict(NCH=NCH, GW=GW, meta=meta,
                    Ktot=[int(k) for k in Ktot],
                    DREL=DREL_t, DRELB=DRELB, IDXW=IDXW)

    L1 = build_layer(src)
    L2 = build_layer(g2row)

    # per-slot block-node gather indices for alpha_dst (layer 1 only)
    BLKI = np.zeros((NCORES, P, NBLK), dtype=np.int32)
    for c in range(NCORES):
        for s in range(NBLK):
            nodes = np.minimum(c * NPC + s * P + np.arange(P), N - 1)
            BLKI[c, :, s] = nodes

    xT = np.zeros((D, NROW1), dtype=np.float32)
    xT[:, :N] = x.T
    A1 = np.zeros((F1, 6), dtype=np.float32)
    for h in range(H):
        A1[h * HID:(h + 1) * HID, h] = as1[h]
        A1[h * HID:(h + 1) * HID, 3 + h] = ad1[h]
    A2 = np.stack([as2[0], ad2[0]], axis=1).astype(np.float32)

    shared = {
        "xT": xT,
        "W1": np.ascontiguousarray(W1),
        "W1T": np.ascontiguousarray(W1.T),
        "A1": A1,
        "W2": np.ascontiguousarray(W2),
        "W2T": np.ascontiguousarray(W2.T),
        "A2": A2,
        "B1": np.ascontiguousarray(np.broadcast_to(b1, (P, F1))),
        "B2": np.ascontiguousarray(np.broadcast_to(b2, (P, F2))),
        "IOTA": np.ascontiguousarray(np.broadcast_to(
            np.arange(P, dtype=ml_dtypes.bfloat16), (P, P))),
        "IOTAC": np.arange(P, dtype=np.float32).reshape(P, 1),
    }
    percore = []
    for c in range(NCORES):
        percore.append({
            "DREL1": L1["DREL"][c], "DRELB1": L1["DRELB"][c],
            "IDXW1": L1["IDXW"][c],
            "DREL2": L2["DREL"][c], "DRELB2": L2["DRELB"][c],
            "IDXW2": L2["IDXW"][c],
            "BLKI": BLKI[c],
        })
    key = (tuple(L1["Ktot"]), tuple(L2["Ktot"]))
    return key, (L1, L2), shared, percore


def _ap_view(ap, extra_offset, free_dims):
    import concourse.bass as bass

    return bass.AP(
        tensor=ap.tensor, offset=ap.offset + extra_offset,
        ap=[list(ap.ap[0])] + [list(d) for d in free_dims],
    )


def _dram_ap(dt_handle, offset, dims):
    """Build a DRAM AP with explicit [stride, num] dims (partition dim first)."""
    import concourse.bass as bass

    ap = dt_handle.ap()
    return bass.AP(tensor=ap.tensor, offset=offset,
                   ap=[list(d) for d in dims])


def _build(L1, L2):
    import concourse.bass as bass
    import concourse.bacc as bacc
    import concourse.tile as tile
    from concourse import mybir
    from concourse.masks import make_identity
    from concourse.library_config import mlp
    from contextlib import ExitStack

    f32 = mybir.dt.float32
    bf16 = mybir.dt.bfloat16
    i32 = mybir.dt.int32
    i16 = mybir.dt.int16
    u8 = mybir.dt.uint8
    AT = mybir.ActivationFunctionType
    OP = mybir.AluOpType
    IOA = bass.IndirectOffsetOnAxis

    nc = bacc.Bacc("TRN2", target_bir_lowering=False, debug=False,
                   num_devices=NCORES, num_swdge_queues=4)

    NCH1, NCH2 = L1["NCH"], L2["NCH"]
    GW1, GW2 = L1["GW"], L2["GW"]

    xT = nc.dram_tensor("xT", [D, NROW1], f32, kind="ExternalInput")
    W1 = nc.dram_tensor("W1", [D, F1], f32, kind="ExternalInput")
    W1T = nc.dram_tensor("W1T", [F1, D], f32, kind="ExternalInput")
    A1 = nc.dram_tensor("A1", [F1, 6], f32, kind="ExternalInput")
    W2 = nc.dram_tensor("W2", [F1, F2], f32, kind="ExternalInput")
    W2T = nc.dram_tensor("W2T", [F2, F1], f32, kind="ExternalInput")
    A2 = nc.dram_tensor("A2", [F2, 2], f32, kind="ExternalInput")
    B1 = nc.dram_tensor("B1", [P, F1], f32, kind="ExternalInput")
    B2 = nc.dram_tensor("B2", [P, F2], f32, kind="ExternalInput")
    IOTA = nc.dram_tensor("IOTA", [P, P], bf16, kind="ExternalInput")
    IOTAC = nc.dram_tensor("IOTAC", [P, 1], f32, kind="ExternalInput")
    DREL1 = nc.dram_tensor("DREL1", [P, NCH1], bf16, kind="ExternalInput")
    DRELB1 = nc.dram_tensor("DRELB1", [P, NCH1 * P], u8, kind="ExternalInput")
    IDXW1 = nc.dram_tensor("IDXW1", [P, NCH1 * 8], i16, kind="ExternalInput")
    DREL2 = nc.dram_tensor("DREL2", [P, NCH2], bf16, kind="ExternalInput")
    DRELB2 = nc.dram_tensor("DRELB2", [P, NCH2 * P], u8, kind="ExternalInput")
    IDXW2 = nc.dram_tensor("IDXW2", [P, NCH2 * 8], i16, kind="ExternalInput")
    BLKI = nc.dram_tensor("BLKI", [P, NBLK], i32, kind="ExternalInput")
    OUT = nc.dram_tensor("out", [NROWC, F2], f32, kind="ExternalOutput")

    G1a = nc.dram_tensor("G1a", [HALF, G1W], bf16, kind="Internal")
    G1b = nc.dram_tensor("G1b", [NROW1 - HALF, G1W], bf16, kind="Internal")
    AD1 = nc.dram_tensor("AD1", [NROW1, 4], f32, kind="Internal")
    G2L = nc.dram_tensor("G2L", [NROWC, G2W], bf16, kind="Internal")
    G2F = nc.dram_tensor("G2F", [NROWC * NCORES, G2W], bf16,
                         addr_space="Shared", kind="Internal")

    with tile.TileContext(nc) as tc, ExitStack() as ctx:
        consts = ctx.enter_context(tc.tile_pool(name="consts", bufs=1))
        sbA = ctx.enter_context(tc.tile_pool(name="sbA", bufs=3))
        pas = ctx.enter_context(tc.tile_pool(name="pas", bufs=2, space="PSUM"))
        pse = ctx.enter_context(tc.tile_pool(name="pse", bufs=2, space="PSUM"))
        psad = ctx.enter_context(tc.tile_pool(name="psad", bufs=2, space="PSUM"))
        pst = ctx.enter_context(tc.tile_pool(name="pst", bufs=1, space="PSUM"))
        pge = ctx.enter_context(tc.tile_pool(name="pge", bufs=1, space="PSUM"))
        gpool = ctx.enter_context(tc.tile_pool(name="gpool", bufs=4))
        fpool = ctx.enter_context(tc.tile_pool(name="fpool", bufs=4))
        spool = ctx.enter_context(tc.tile_pool(name="spool", bufs=4))
        ipool = ctx.enter_context(tc.tile_pool(name="ipool", bufs=4))
        epool = ctx.enter_context(tc.tile_pool(name="epool", bufs=4))

        nc.gpsimd.load_library(mlp)

        # ---------------- constants / preloads ----------------
        iotab = consts.tile([P, P], bf16)
        nc.sync.dma_start(out=iotab[:], in_=IOTA[:])
        iotac = consts.tile([P, 1], f32)
        nc.sync.dma_start(out=iotac[:], in_=IOTAC[:])
        b1t = consts.tile([P, F1], f32)
        nc.sync.dma_start(out=b1t[:], in_=B1[:])
        b2t = consts.tile([P, F2], f32)
        nc.sync.dma_start(out=b2t[:], in_=B2[:])
        ident = consts.tile([P, P], f32)
        make_identity(nc, ident[:])

        idxall1 = consts.tile([P, NCH1 * 8], i16)
        nc.sync.dma_start(out=idxall1[:], in_=IDXW1[:])
        drelall1 = consts.tile([P, NCH1], bf16)
        nc.sync.dma_start(out=drelall1[:], in_=DREL1[:])
        idxall2 = consts.tile([P, NCH2 * 8], i16)
        nc.sync.dma_start(out=idxall2[:], in_=IDXW2[:])
        drelall2 = consts.tile([P, NCH2], bf16)
        nc.sync.dma_start(out=drelall2[:], in_=DREL2[:])
        blkit = consts.tile([P, NBLK], i32)
        nc.sync.dma_start(out=blkit[:], in_=BLKI[:])

        rhs1 = consts.tile([P, 198], f32)
        nc.sync.dma_start(out=rhs1[:, :F1], in_=W1[:])
        w1t_a = consts.tile([P, D], f32)
        nc.sync.dma_start(out=w1t_a[:], in_=W1T[0:P, :])
        w1t_b = consts.tile([F1 - P, D], f32)
        nc.sync.dma_start(out=w1t_b[:], in_=W1T[P:F1, :])
        a1_a = consts.tile([P, 6], f32)
        nc.sync.dma_start(out=a1_a[:], in_=A1[0:P, :])
        a1_b = consts.tile([F1 - P, 6], f32)
        nc.sync.dma_start(out=a1_b[:], in_=A1[P:F1, :])
        pu = pge.tile([P, 200], f32, tag="pg")
        nc.tensor.matmul(out=pu[:, :6], lhsT=w1t_a[:], rhs=a1_a[:],
                         start=True, stop=False)
        nc.tensor.matmul(out=pu[:, :6], lhsT=w1t_b[:], rhs=a1_b[:],
                         start=False, stop=True)
        nc.vector.tensor_copy(out=rhs1[:, F1:F1 + 6], in_=pu[:, :6])

        w2t = consts.tile([F2, F1], f32)
        nc.sync.dma_start(out=w2t[:], in_=W2T[:])
        a2t = consts.tile([F2, 2], f32)
        nc.sync.dma_start(out=a2t[:], in_=A2[:])
        rhs2_lo = consts.tile([P, 66], f32)
        nc.sync.dma_start(out=rhs2_lo[:, :F2], in_=W2[0:P, :])
        rhs2_hi = consts.tile([F1 - P, 66], f32)
        nc.sync.dma_start(out=rhs2_hi[:, :F2], in_=W2[P:F1, :])
        pu2 = pst.tile([P, 256], f32, tag="tr")
        nc.tensor.matmul(out=pu2[:, :2], lhsT=w2t[:, 0:P], rhs=a2t[:],
                         start=True, stop=True)
        nc.vector.tensor_copy(out=rhs2_lo[:, F2:F2 + 2], in_=pu2[:, :2])
        pu2b = pst.tile([P, 256], f32, tag="tr")
        nc.tensor.matmul(out=pu2b[:F1 - P, 4:6], lhsT=w2t[:, P:F1], rhs=a2t[:],
                         start=True, stop=True)
        nc.vector.tensor_copy(out=rhs2_hi[:, F2:F2 + 2], in_=pu2b[:F1 - P, 4:6])

        # alpha_dst-block tables (slot-major, partition = node-within-slot)
        adbh1 = consts.tile([P, NBLK, 4], bf16)
        adb2_sb = consts.tile([P, NBLK, 1], bf16)

        # ---------------- stage A (G1b tiles first), batched ----------------
        def stage_a_group(t0, nb):
            xt = sbA.tile([P, BA * P], f32, tag="xt")
            nc.sync.dma_start(out=xt[:, :nb * P],
                              in_=xT[:, t0 * P:(t0 + nb) * P])
            gbf = sbA.tile([P, BA, G1W], bf16, tag="gbf")
            gf32 = gbf[:].bitcast(f32)
            nc.vector.memset(gf32[:, :, 99:128], 0.0)
            adw = sbA.tile([P, BA, 4], f32, tag="adw")
            for b in range(nb):
                pa = pas.tile([P, 200], f32, tag="pa")
                nc.tensor.matmul(out=pa[:, :198], lhsT=xt[:, b * P:(b + 1) * P],
                                 rhs=rhs1[:], start=True, stop=True)
                nc.scalar.activation(out=gbf[:, b, :F1], in_=pa[:, :F1],
                                     func=AT.Copy)
                nc.vector.tensor_copy(out=gf32[:, b, 96:99],
                                      in_=pa[:, F1:F1 + 3])
                nc.vector.tensor_copy(out=adw[:, b, :3], in_=pa[:, F1 + 3:F1 + 6])
            if t0 >= HALF // P:
                tb0 = t0 - HALF // P
                dst_ap = _dram_ap(G1b, tb0 * P * G1W,
                                  [[G1W, P], [P * G1W, nb], [1, G1W]])
            else:
                dst_ap = _dram_ap(G1a, t0 * P * G1W,
                                  [[G1W, P], [P * G1W, nb], [1, G1W]])
            nc.scalar.dma_start(out=dst_ap, in_=gbf[:, :nb, :])
            ad_ap = _dram_ap(AD1, t0 * P * 4, [[4, P], [P * 4, nb], [1, 3]])
            nc.sync.dma_start(out=ad_ap, in_=adw[:, :nb, :3])

        tiles_b = list(range(HALF // P, NT))
        tiles_a = list(range(HALF // P))
        for tl in (tiles_b, tiles_a):
            i = 0
            while i < len(tl):
                nb = min(BA, len(tl) - i)
                # groups must stay within one table and contiguous
                stage_a_group(tl[i], nb)
                i += nb

        # L1 alpha_dst blocks: per-slot indirect gathers, then one bf16 convert
        adb1f = consts.tile([P, NBLK, 4], f32)
        for s in range(NBLK):
            nc.gpsimd.indirect_dma_start(
                out=adb1f[:, s, :], out_offset=None, in_=AD1[:],
                in_offset=IOA(ap=blkit[:, s:s + 1], axis=0))
        nc.vector.tensor_copy(out=adbh1[:], in_=adb1f[:])

        # ---------------- generic edge phase (one slot per iteration) -------
        def edge_layer(LM, TBLa, TBLb, width, nfeat, as_f32col, adbh, nheads,
                       drelall, drelbx, idxall, ps_width, slot_epilogue):
            meta = LM["meta"]
            GW = LM["GW"]
            fw = nfeat + nheads
            hd = nfeat // nheads
            for s in range(NBLK):
                c0, kb, ka = meta[s]
                nch = kb + ka
                drelb_t = ipool.tile([P, GW * P], u8, tag="drelb",
                                     name="drelb_t")
                nc.sync.dma_start(out=drelb_t[:, :nch * P],
                                  in_=drelbx[:, c0 * P:(c0 + nch) * P])
                grow = gpool.tile([P, GW, width], bf16, tag="grow", name="grow")
                # all gathers on queue 0: DMASW sem lanes are assigned by Tile
                # in scheduled order (unpredictable here); a single queue keeps
                # every sem's updater queue consistent. <=8 chunks (1024 idxs)
                # per gather op (ucode limit).
                for tbl, lo, cnt in ((TBLb, 0, kb), (TBLa, kb, ka)):
                    j = 0
                    while j < cnt:
                        nj = min(8, cnt - j)
                        cj = c0 + lo + j
                        nc.gpsimd.dma_gather(
                            grow[:, lo + j:lo + j + nj, :], tbl[:],
                            idxall[:, cj * 8:(cj + nj) * 8], nj * P, nj * P,
                            width, queue_num=0)
                        j += nj
                # S8: [e_part, chunk, d] one-hot (bf16 all the way)
                S8 = spool.tile([P, GW * P], bf16, tag="s8", name="s8")
                nc.vector.tensor_tensor(
                    out=_ap_view(S8[:], 0, [[P, nch], [1, P]]),
                    in0=_ap_view(drelall[:], c0, [[1, nch], [0, P]]),
                    in1=_ap_view(iotab[:], 0, [[0, nch], [1, P]]),
                    op=OP.is_equal)
                # S_T: [d_part, chunk*P + e] one-hot from u8 drel broadcast
                st8 = spool.tile([P, GW * P], bf16, tag="st8", name="st8")
                nc.vector.tensor_scalar(
                    out=st8[:, :nch * P], in0=drelb_t[:, :nch * P],
                    scalar1=iotac[:, :1], scalar2=None, op0=OP.is_equal)
                # alpha_dst expansion: adp[e, ch*nheads+h]
                ps = pse.tile([P, 200], f32, tag="ps", name="ps")
                adp = psad.tile([P, GW * nheads], f32, tag="adp", name="adp")
                for jj in range(nch):
                    nc.tensor.matmul(
                        out=adp[:, jj * nheads:(jj + 1) * nheads],
                        lhsT=st8[:, jj * P:(jj + 1) * P],
                        rhs=adbh[:, s, :nheads],
                        start=True, stop=True)
                # logits -> exp -> weighted features
                growf = grow[:].bitcast(f32)
                t8 = epool.tile([P, GW * nheads], f32, tag="t8", name="t8")
                nc.vector.tensor_tensor(
                    out=_ap_view(t8[:], 0, [[nheads, nch], [1, nheads]]),
                    in0=_ap_view(growf, as_f32col,
                                 [[width // 2, nch], [1, nheads]]),
                    in1=_ap_view(adp[:], 0, [[nheads, nch], [1, nheads]]),
                    op=OP.add)
                # exp(lrelu(t)) == max(exp(t), exp(SLOPE*t)) exactly
                e2 = epool.tile([P, GW * nheads], f32, tag="e2", name="e2")
                nc.scalar.activation(out=e2[:, :nch * nheads],
                                     in_=t8[:, :nch * nheads],
                                     func=AT.Exp, scale=SLOPE)
                F8 = fpool.tile([P, GW * fw], bf16, tag="f8", name="f8")
                nc.scalar.activation(
                    out=_ap_view(F8[:], nfeat, [[fw, nch], [1, nheads]]),
                    in_=_ap_view(t8[:], 0, [[nheads, nch], [1, nheads]]),
                    func=AT.Exp)
                nc.vector.tensor_tensor(
                    out=_ap_view(F8[:], nfeat, [[fw, nch], [1, nheads]]),
                    in0=_ap_view(F8[:], nfeat, [[fw, nch], [1, nheads]]),
                    in1=_ap_view(e2[:], 0, [[nheads, nch], [1, nheads]]),
                    op=OP.max)
                nc.vector.tensor_tensor(
                    out=_ap_view(F8[:], 0, [[fw, nch], [hd, nheads], [1, hd]]),
                    in0=_ap_view(grow[:], 0,
                                 [[width, nch], [hd, nheads], [1, hd]]),
                    in1=_ap_view(F8[:], nfeat,
                                 [[fw, nch], [1, nheads], [0, hd]]),
                    op=OP.mult)
                for jj in range(nch):
                    nc.tensor.matmul(
                        out=ps[:, :ps_width],
                        lhsT=S8[:, jj * P:(jj + 1) * P],
                        rhs=F8[:, jj * fw:jj * fw + ps_width],
                        start=(jj == 0), stop=(jj == nch - 1))
                slot_epilogue(s, ps)

        # L1 epilogue: h -> transpose -> G2 rows + ad2 (SBUF)
        def epi1(s, ps):
            rc = epool.tile([P, H], f32, tag="rc", name="rc")
            nc.vector.tensor_scalar_add(out=rc[:], in0=ps[:, F1:F1 + H],
                                        scalar1=EPS)
            rc2 = epool.tile([P, H], f32, tag="rc2", name="rc2")
            nc.vector.reciprocal(out=rc2[:], in_=rc[:])
            hm = epool.tile([P, F1], f32, tag="hm", name="hm")
            nc.vector.tensor_tensor(
                out=_ap_view(hm[:], 0, [[HID, H], [1, HID]]),
                in0=_ap_view(ps[:, :F1], 0, [[HID, H], [1, HID]]),
                in1=_ap_view(rc2[:], 0, [[1, H], [0, HID]]),
                op=OP.mult)
            hb = epool.tile([P, F1], f32, tag="hb", name="hb")
            nc.vector.tensor_tensor(out=hb[:], in0=hm[:], in1=b1t[:], op=OP.add)
            hr = epool.tile([P, F1], f32, tag="hr", name="hr")
            nc.scalar.activation(out=hr[:], in_=hb[:], func=AT.Relu)
            pt = pst.tile([P, 256], f32, tag="tr", name="pt")
            nc.tensor.transpose(out=pt[:, 0:P], in_=hr[:, :P], identity=ident[:])
            nc.tensor.transpose(out=pt[:F1 - P, P:256], in_=hr[:, P:F1],
                                identity=ident[:])
            ht1 = epool.tile([P, P], f32, tag="ht1", name="ht1")
            nc.vector.tensor_copy(out=ht1[:], in_=pt[:, 0:P])
            ht2 = epool.tile([F1 - P, P], f32, tag="ht2", name="ht2")
            nc.vector.tensor_copy(out=ht2[:], in_=pt[:F1 - P, P:256])
            pg = pge.tile([P, 200], f32, tag="pg", name="pg")
            nc.tensor.matmul(out=pg[:, :66], lhsT=ht1[:], rhs=rhs2_lo[:],
                             start=True, stop=False)
            nc.tensor.matmul(out=pg[:, :66], lhsT=ht2[:], rhs=rhs2_hi[:],
                             start=False, stop=True)
            g2 = epool.tile([P, G2W], bf16, tag="g2", name="g2")
            nc.vector.tensor_copy(out=g2[:, :F2], in_=pg[:, :F2])
            g2f = g2[:].bitcast(f32)
            nc.vector.memset(g2f[:, 33:64], 0.0)
            nc.vector.tensor_copy(out=g2f[:, 32:33], in_=pg[:, F2:F2 + 1])
            nc.vector.tensor_copy(out=adb2_sb[:, s, :1], in_=pg[:, F2 + 1:F2 + 2])
            nc.sync.dma_start(out=G2L[s * P:(s + 1) * P, :], in_=g2[:])

        edge_layer(L1, G1a, G1b, G1W, F1, 96, adbh1, H,
                   drelall1, DRELB1, idxall1, F1 + H, epi1)

        # ---------------- AllGather (G2 only) ----------------
        nc.gpsimd.collective_compute(
            "AllGather", mybir.AluOpType.bypass,
            replica_groups=[list(range(NCORES))],
            ins=[G2L.ap().opt()], outs=[G2F.ap().opt()])

        # ---------------- layer 2 ----------------
        def epi2(s, ps):
            rc = epool.tile([P, 1], f32, tag="rcB", name="rcB")
            nc.vector.tensor_scalar_add(out=rc[:], in0=ps[:, F2:F2 + 1],
                                        scalar1=EPS)
            rc2 = epool.tile([P, 1], f32, tag="rcB2", name="rcB2")
            nc.vector.reciprocal(out=rc2[:], in_=rc[:])
            om = epool.tile([P, F2], f32, tag="om", name="om")
            nc.vector.tensor_tensor(out=om[:], in0=ps[:, :F2],
                                    in1=rc2[:].to_broadcast([P, F2]),
                                    op=OP.mult)
            ob = epool.tile([P, F2], f32, tag="ob", name="ob")
            nc.vector.tensor_tensor(out=ob[:], in0=om[:], in1=b2t[:], op=OP.add)
            orl = epool.tile([P, F2], f32, tag="orl", name="orl")
            nc.scalar.activation(out=orl[:], in_=ob[:], func=AT.Relu)
            nc.sync.dma_start(out=OUT[s * P:(s + 1) * P, :], in_=orl[:])

        # G2F views for the two index halves (offsets stay < 2^24 bytes)
        g2fa = G2F[0:HALF, :]
        g2fb = G2F[HALF:NROWC * NCORES, :]
        edge_layer(L2, g2fa, g2fb, G2W, F2, 32, adb2_sb, 1,
                   drelall2, DRELB2, idxall2, F2 + 1, epi2)

    nc.compile()
    return nc


def _get_compiled(key, layers):
    if key not in _compiled:
        _compiled[key] = _build(layers[0], layers[1])
    return _compiled[key]


def run(inputs, **runkw):
    from concourse import bass_utils

    key, layers, shared, percore = _host_prep(inputs)
    nc = _get_compiled(key, layers)
    in_maps = []
    for c in range(NCORES):
        m = dict(shared)
        m.update(percore[c])
        in_maps.append(m)
    res = bass_utils.run_bass_kernel_spmd(
        nc, in_maps, core_ids=list(range(NCORES)), **runkw)
    return res


def assemble(results):
    out = np.empty((N, F2), dtype=np.float32)
    for c in range(NCORES):
        out[c * NPC:(c + 1) * NPC] = results[c]["out"][:NPC]
    return out


def kernel(**inputs):
    res = run(inputs)
    return assemble(res.results)


# revision 24
# speedup vs baseline: 1.6578x; 1.6578x over previous
"""GAT 2-layer kernel for Trainium2, 8 NeuronCores (SPMD, dst-sharded).

Strategy (v4):
  - Destination-node sharding: core c owns nodes [c*6250,(c+1)*6250); edges bucketed
    into per-128-dst-node "slots" (49/core), padded to 128-edge chunks.
  - Stage A (replicated, batched x4): per 128-node tile one matmul computes
    [x@W1 (192) | alpha_src (3) | alpha_dst (3)]; xw+as go to a bf16 gather table
    G1 (512B rows); ad to slim f32 table AD1. Loads/stores batched 4 tiles per DMA
    to cut sequencer descriptor-generation. G1 split into two <=32768-row tensors
    (dma_gather int16 index limit).
  - Edge phase per layer, one SLOT (<=17 chunks) per iteration: two dma_gathers
    (one per table half) pull all source rows of the slot into one grow tile;
    indices/drel preloaded to SBUF once (no per-op JIT index DMAs -> no gather
    stalls). One-hot S (DVE is_equal vs iota, bf16) segment-reduces exp-weighted
    features AND softmax denominators per-slot in PSUM. alpha_dst[dst] expanded
    edge-wise via host-precomputed u8 drel-broadcast (DRELB) + DVE is_equal
    (replaces the PE ones-matmul broadcast) -> per-chunk S_T matmul vs the slot's
    alpha_dst block (preloaded: one batched indirect gather for L1; SBUF-resident
    for L2 -- no AD2 AllGather).
  - Per-slot epilogue: h = relu(sum/(denom+eps) + bias1); PE-transpose h, emit G2
    rows [h@W2 (64) bf16 | as2 f32] to DRAM and ad2 to SBUF; AllGather G2 only;
    layer 2 repeats the edge phase (1 head) against G2F views.
"""
import sys

sys.path.insert(0, "/opt/trn_rl_repo")
import numpy as np
import ml_dtypes

N = 50000
D = 128
HID = 64
H = 3
F1 = 192
F2 = 64
NCORES = 8
NPC = N // NCORES          # 6250 nodes per core
P = 128
NBLK = (NPC + P - 1) // P  # 49 slots per core
NT = (N + P - 1) // P      # 391 stage-A node tiles
NROW1 = NT * P             # 50048 G1 rows
HALF = 32768               # dma_gather int16 index limit
G1W = 256                  # bf16 cols: xw(192) | as f32 x3 (bf16 192:198) | pad
G2W = 128                  # bf16 cols: xw2(64) | as2 f32 (bf16 64:66) | pad
NROWC = NBLK * P           # 6272 rows per core shard
SLOPE = 0.2
EPS = 1e-16
BA = 4                     # stage-A tiles per DMA batch
GWMAX = 24                 # max chunks per slot supported by iota consts

_compiled = {}


def _host_prep(inputs):
    x = np.asarray(inputs["x"], dtype=np.float32)
    ei = np.asarray(inputs["edge_index"])
    W1 = np.asarray(inputs["W1"], dtype=np.float32)
    as1 = np.asarray(inputs["att_src1"], dtype=np.float32)
    ad1 = np.asarray(inputs["att_dst1"], dtype=np.float32)
    b1 = np.asarray(inputs["bias1"], dtype=np.float32)
    W2 = np.asarray(inputs["W2"], dtype=np.float32)
    as2 = np.asarray(inputs["att_src2"], dtype=np.float32)
    ad2 = np.asarray(inputs["att_dst2"], dtype=np.float32)
    b2 = np.asarray(inputs["bias2"], dtype=np.float32)

    loops = np.arange(N, dtype=np.int64)
    src = np.concatenate([ei[0].astype(np.int64), loops])
    dst = np.concatenate([ei[1].astype(np.int64), loops])
    order = np.argsort(dst, kind="stable")
    src = src[order]
    dst = dst[order]
    g2row = (src // NPC) * NROWC + (src % NPC)

    def build_layer(skey):
        # per-slot chunk counts (max over cores so program is uniform)
        core = dst // NPC
        rel = dst % NPC
        slot = rel // P
        half = (skey >= HALF).astype(np.int64)
        counts = np.zeros((NCORES, NBLK, 2), dtype=np.int64)
        np.add.at(counts, (core, slot, half), 1)
        Ka = np.ceil(counts[:, :, 0] / P).astype(np.int64).max(axis=0)
        Kb = np.ceil(counts[:, :, 1] / P).astype(np.int64).max(axis=0)
        Ktot = Ka + Kb
        NCH = int(Ktot.sum())
        GW = int(Ktot.max())
        # meta per slot: (chunk_base, Kb, Ka); b-chunks first within a slot
        meta = []
        cb = 0
        for s in range(NBLK):
            meta.append((cb, int(Kb[s]), int(Ka[s])))
            cb += int(Kb[s] + Ka[s])

        EPAD = NCH * P
        SRCK = np.zeros((NCORES, EPAD), dtype=np.int64)
        DREL = np.full((NCORES, EPAD), 255.0, dtype=np.float32)
        for c in range(NCORES):
            base_node = c * NPC
            cb = 0
            for s in range(NBLK):
                blo = base_node + s * P
                bhi = min(blo + P, base_node + NPC)
                lo = np.searchsorted(dst, blo, side="left")
                hi = np.searchsorted(dst, bhi, side="left")
                sk = skey[lo:hi]
                dr = (dst[lo:hi] - blo).astype(np.float32)
                a_mask = sk < HALF
                for which, KK, pad in ((~a_mask, Kb[s], HALF),
                                       (a_mask, Ka[s], 0)):
                    cnt = int(which.sum())
                    pos = cb * P
                    SRCK[c, pos:pos + cnt] = sk[which]
                    SRCK[c, pos + cnt:(cb + int(KK)) * P] = pad
                    DREL[c, pos:pos + cnt] = dr[which]
                    cb += int(KK)
        # device arrays
        DREL_t = np.ascontiguousarray(
            DREL.reshape(NCORES, NCH, P).transpose(0, 2, 1)
        ).astype(ml_dtypes.bfloat16)                       # [C, P, NCH]
        DRELB = np.ascontiguousarray(np.broadcast_to(
            DREL.astype(np.uint8).reshape(NCORES, 1, EPAD),
            (NCORES, P, EPAD)).transpose(1, 0, 2)).transpose(1, 0, 2)
        # ^ [C, P, NCH*P] u8, rows identical per partition
        # wrapped int16 indices, per chunk 8 cols: [P, NCH*8]
        IDXW = np.zeros((NCORES, P, NCH * 8), dtype=np.int16)
        for c in range(NCORES):
            for s in range(NBLK):
                c0, kb, ka = meta[s]
                for part, nch_p, base in ((0, kb, HALF), (kb, ka, 0)):
                    if nch_p == 0:
                        continue
                    lo = (c0 + part) * P
                    iv = SRCK[c, lo:lo + nch_p * P] - base
                    w = iv.reshape(-1, 16).T.astype(np.int16)  # [16, n/16]
                    IDXW[c, :, (c0 + part) * 8:(c0 + part + nch_p) * 8] = \
                        np.tile(w, (8, 1))
        return dict(NCH=NCH, GW=GW, meta=meta,
                    Ktot=[int(k) for k in Ktot],
                    DREL=DREL_t, DRELB=DRELB, IDXW=IDXW)

    L1 = build_layer(src)
    L2 = build_layer(g2row)

    # per-slot block-node gather indices for alpha_dst (layer 1 only)
    BLKI = np.zeros((NCORES, P, NBLK), dtype=np.int32)
    for c in range(NCORES):
        for s in range(NBLK):
            nodes = np.minimum(c * NPC + s * P + np.arange(P), N - 1)
            BLKI[c, :, s] = nodes

    xT = np.zeros((D, NROW1), dtype=np.float32)
    xT[:, :N] = x.T
    A1 = np.zeros((F1, 6), dtype=np.float32)
    for h in range(H):
        A1[h * HID:(h + 1) * HID, h] = as1[h]
        A1[h * HID:(h + 1) * HID, 3 + h] = ad1[h]
    A2 = np.stack([as2[0], ad2[0]], axis=1).astype(np.float32)

    shared = {
        "xT": xT,
        "W1": np.ascontiguousarray(W1),
        "W1T": np.ascontiguousarray(W1.T),
        "A1": A1,
        "W2": np.ascontiguousarray(W2),
        "W2T": np.ascontiguousarray(W2.T),
        "A2": A2,
        "B1": np.ascontiguousarray(np.broadcast_to(b1, (P, F1))),
        "B2": np.ascontiguousarray(np.broadcast_to(b2, (P, F2))),
        "IOTAROW": np.ascontiguousarray(np.broadcast_to(
            np.tile(np.arange(P, dtype=ml_dtypes.bfloat16), GWMAX),
            (P, GWMAX * P))),
        "IOTACOL": np.ascontiguousarray(np.broadcast_to(
            np.arange(P, dtype=np.uint8).reshape(P, 1), (P, GWMAX * P))),
    }
    percore = []
    for c in range(NCORES):
        percore.append({
            "DREL1": L1["DREL"][c], "DRELB1": L1["DRELB"][c],
            "IDXW1": L1["IDXW"][c],
            "DREL2": L2["DREL"][c], "DRELB2": L2["DRELB"][c],
            "IDXW2": L2["IDXW"][c],
            "BLKI": BLKI[c],
        })
    key = (tuple(L1["Ktot"]), tuple(L2["Ktot"]))
    return key, (L1, L2), shared, percore


def _ap_view(ap, extra_offset, free_dims):
    import concourse.bass as bass

    return bass.AP(
        tensor=ap.tensor, offset=ap.offset + extra_offset,
        ap=[list(ap.ap[0])] + [list(d) for d in free_dims],
    )


def _dram_ap(dt_handle, offset, dims):
    """Build a DRAM AP with explicit [stride, num] dims (partition dim first)."""
    import concourse.bass as bass

    ap = dt_handle.ap()
    return bass.AP(tensor=ap.tensor, offset=offset,
                   ap=[list(d) for d in dims])


def _build(L1, L2):
    import os
    global _SIM_Q0
    _SIM_Q0 = bool(int(os.environ.get("BASS_SIM_Q0", "0")))
    import concourse.bass as bass
    import concourse.bacc as bacc
    import concourse.tile as tile
    from concourse import mybir
    from concourse.masks import make_identity
    from concourse.library_config import mlp
    from contextlib import ExitStack

    f32 = mybir.dt.float32
    bf16 = mybir.dt.bfloat16
    i32 = mybir.dt.int32
    i16 = mybir.dt.int16
    u8 = mybir.dt.uint8
    AT = mybir.ActivationFunctionType
    OP = mybir.AluOpType
    IOA = bass.IndirectOffsetOnAxis

    nc = bacc.Bacc("TRN2", target_bir_lowering=False, debug=False,
                   num_devices=NCORES, num_swdge_queues=4)

    NCH1, NCH2 = L1["NCH"], L2["NCH"]
    GW1, GW2 = L1["GW"], L2["GW"]

    xT = nc.dram_tensor("xT", [D, NROW1], f32, kind="ExternalInput")
    W1 = nc.dram_tensor("W1", [D, F1], f32, kind="ExternalInput")
    W1T = nc.dram_tensor("W1T", [F1, D], f32, kind="ExternalInput")
    A1 = nc.dram_tensor("A1", [F1, 6], f32, kind="ExternalInput")
    W2 = nc.dram_tensor("W2", [F1, F2], f32, kind="ExternalInput")
    W2T = nc.dram_tensor("W2T", [F2, F1], f32, kind="ExternalInput")
    A2 = nc.dram_tensor("A2", [F2, 2], f32, kind="ExternalInput")
    B1 = nc.dram_tensor("B1", [P, F1], f32, kind="ExternalInput")
    B2 = nc.dram_tensor("B2", [P, F2], f32, kind="ExternalInput")
    IOTAROW = nc.dram_tensor("IOTAROW", [P, GWMAX * P], bf16,
                             kind="ExternalInput")
    IOTACOL = nc.dram_tensor("IOTACOL", [P, GWMAX * P], u8,
                             kind="ExternalInput")
    DREL1 = nc.dram_tensor("DREL1", [P, NCH1], bf16, kind="ExternalInput")
    DRELB1 = nc.dram_tensor("DRELB1", [P, NCH1 * P], u8, kind="ExternalInput")
    IDXW1 = nc.dram_tensor("IDXW1", [P, NCH1 * 8], i16, kind="ExternalInput")
    DREL2 = nc.dram_tensor("DREL2", [P, NCH2], bf16, kind="ExternalInput")
    DRELB2 = nc.dram_tensor("DRELB2", [P, NCH2 * P], u8, kind="ExternalInput")
    IDXW2 = nc.dram_tensor("IDXW2", [P, NCH2 * 8], i16, kind="ExternalInput")
    BLKI = nc.dram_tensor("BLKI", [P, NBLK], i32, kind="ExternalInput")
    OUT = nc.dram_tensor("out", [NROWC, F2], f32, kind="ExternalOutput")

    G1a = nc.dram_tensor("G1a", [HALF, G1W], bf16, kind="Internal")
    G1b = nc.dram_tensor("G1b", [NROW1 - HALF, G1W], bf16, kind="Internal")
    AD1 = nc.dram_tensor("AD1", [NROW1, 4], f32, kind="Internal")
    G2L = nc.dram_tensor("G2L", [NROWC, G2W], bf16, kind="Internal")
    G2F = nc.dram_tensor("G2F", [NROWC * NCORES, G2W], bf16,
                         addr_space="Shared", kind="Internal")

    with tile.TileContext(nc) as tc, ExitStack() as ctx:
        consts = ctx.enter_context(tc.tile_pool(name="consts", bufs=1))
        sbA = ctx.enter_context(tc.tile_pool(name="sbA", bufs=3))
        pas = ctx.enter_context(tc.tile_pool(name="pas", bufs=3, space="PSUM"))
        pse = ctx.enter_context(tc.tile_pool(name="pse", bufs=2, space="PSUM"))
        psad = ctx.enter_context(tc.tile_pool(name="psad", bufs=1, space="PSUM"))
        pst = ctx.enter_context(tc.tile_pool(name="pst", bufs=1, space="PSUM"))
        pge = ctx.enter_context(tc.tile_pool(name="pge", bufs=1, space="PSUM"))
        gpool = ctx.enter_context(tc.tile_pool(name="gpool", bufs=6))
        fpool = ctx.enter_context(tc.tile_pool(name="fpool", bufs=4))
        spool = ctx.enter_context(tc.tile_pool(name="spool", bufs=5))
        ipool = ctx.enter_context(tc.tile_pool(name="ipool", bufs=6))
        epool = ctx.enter_context(tc.tile_pool(name="epool", bufs=4))

        nc.gpsimd.load_library(mlp)

        # ---------------- constants / preloads ----------------
        iotarow = consts.tile([P, GWMAX * P], bf16)
        nc.sync.dma_start(out=iotarow[:], in_=IOTAROW[:])
        iotacol = consts.tile([P, GWMAX * P], u8)
        nc.sync.dma_start(out=iotacol[:], in_=IOTACOL[:])
        b1t = consts.tile([P, F1], f32)
        nc.sync.dma_start(out=b1t[:], in_=B1[:])
        b2t = consts.tile([P, F2], f32)
        nc.sync.dma_start(out=b2t[:], in_=B2[:])
        ident = consts.tile([P, P], f32)
        make_identity(nc, ident[:])

        idxall1 = consts.tile([P, NCH1 * 8], i16)
        nc.sync.dma_start(out=idxall1[:], in_=IDXW1[:])
        drelall1 = consts.tile([P, NCH1], bf16)
        nc.sync.dma_start(out=drelall1[:], in_=DREL1[:])
        idxall2 = consts.tile([P, NCH2 * 8], i16)
        nc.sync.dma_start(out=idxall2[:], in_=IDXW2[:])
        drelall2 = consts.tile([P, NCH2], bf16)
        nc.sync.dma_start(out=drelall2[:], in_=DREL2[:])
        blkit = consts.tile([P, NBLK], i32)
        nc.sync.dma_start(out=blkit[:], in_=BLKI[:])

        rhs1 = consts.tile([P, 198], f32)
        nc.sync.dma_start(out=rhs1[:, :F1], in_=W1[:])
        w1t_a = consts.tile([P, D], f32)
        nc.sync.dma_start(out=w1t_a[:], in_=W1T[0:P, :])
        w1t_b = consts.tile([F1 - P, D], f32)
        nc.sync.dma_start(out=w1t_b[:], in_=W1T[P:F1, :])
        a1_a = consts.tile([P, 6], f32)
        nc.sync.dma_start(out=a1_a[:], in_=A1[0:P, :])
        a1_b = consts.tile([F1 - P, 6], f32)
        nc.sync.dma_start(out=a1_b[:], in_=A1[P:F1, :])
        pu = pge.tile([P, 200], f32, tag="pg")
        nc.tensor.matmul(out=pu[:, :6], lhsT=w1t_a[:], rhs=a1_a[:],
                         start=True, stop=False)
        nc.tensor.matmul(out=pu[:, :6], lhsT=w1t_b[:], rhs=a1_b[:],
                         start=False, stop=True)
        nc.vector.tensor_copy(out=rhs1[:, F1:F1 + 6], in_=pu[:, :6])

        w2t = consts.tile([F2, F1], f32)
        nc.sync.dma_start(out=w2t[:], in_=W2T[:])
        a2t = consts.tile([F2, 2], f32)
        nc.sync.dma_start(out=a2t[:], in_=A2[:])
        rhs2_lo = consts.tile([P, 66], f32)
        nc.sync.dma_start(out=rhs2_lo[:, :F2], in_=W2[0:P, :])
        rhs2_hi = consts.tile([F1 - P, 66], f32)
        nc.sync.dma_start(out=rhs2_hi[:, :F2], in_=W2[P:F1, :])
        pu2 = pst.tile([P, 256], f32, tag="tr")
        nc.tensor.matmul(out=pu2[:, :2], lhsT=w2t[:, 0:P], rhs=a2t[:],
                         start=True, stop=True)
        nc.vector.tensor_copy(out=rhs2_lo[:, F2:F2 + 2], in_=pu2[:, :2])
        pu2b = pst.tile([P, 256], f32, tag="tr")
        nc.tensor.matmul(out=pu2b[:F1 - P, 4:6], lhsT=w2t[:, P:F1], rhs=a2t[:],
                         start=True, stop=True)
        nc.vector.tensor_copy(out=rhs2_hi[:, F2:F2 + 2], in_=pu2b[:F1 - P, 4:6])

        # alpha_dst-block tables (slot-major, partition = node-within-slot)
        adbh1 = consts.tile([P, NBLK, 4], bf16)
        adb2_sb = consts.tile([P, NBLK, 1], bf16)

        # ---------------- stage A (G1b tiles first), batched ----------------
        def stage_a_group(t0, nb):
            xt = sbA.tile([P, BA * P], f32, tag="xt")
            nc.sync.dma_start(out=xt[:, :nb * P],
                              in_=xT[:, t0 * P:(t0 + nb) * P])
            gbf = sbA.tile([P, BA, G1W], bf16, tag="gbf")
            gf32 = gbf[:].bitcast(f32)
            nc.vector.memset(gf32[:, :, 99:128], 0.0)
            adw = sbA.tile([P, BA, 4], f32, tag="adw")
            for b in range(nb):
                pa = pas.tile([P, 200], f32, tag="pa")
                nc.tensor.matmul(out=pa[:, :198], lhsT=xt[:, b * P:(b + 1) * P],
                                 rhs=rhs1[:], start=True, stop=True)
                nc.scalar.activation(out=gbf[:, b, :F1], in_=pa[:, :F1],
                                     func=AT.Copy)
                nc.vector.tensor_copy(out=gf32[:, b, 96:99],
                                      in_=pa[:, F1:F1 + 3])
                nc.vector.tensor_copy(out=adw[:, b, :3], in_=pa[:, F1 + 3:F1 + 6])
            if t0 >= HALF // P:
                tb0 = t0 - HALF // P
                dst_ap = _dram_ap(G1b, tb0 * P * G1W,
                                  [[G1W, P], [P * G1W, nb], [1, G1W]])
            else:
                dst_ap = _dram_ap(G1a, t0 * P * G1W,
                                  [[G1W, P], [P * G1W, nb], [1, G1W]])
            nc.scalar.dma_start(out=dst_ap, in_=gbf[:, :nb, :])
            ad_ap = _dram_ap(AD1, t0 * P * 4, [[4, P], [P * 4, nb], [1, 3]])
            nc.sync.dma_start(out=ad_ap, in_=adw[:, :nb, :3])

        tiles_b = list(range(HALF // P, NT))
        tiles_a = list(range(HALF // P))
        for tl in (tiles_b, tiles_a):
            i = 0
            while i < len(tl):
                nb = min(BA, len(tl) - i)
                # groups must stay within one table and contiguous
                stage_a_group(tl[i], nb)
                i += nb

        # L1 alpha_dst blocks: per-slot indirect gathers, then one bf16 convert
        adb1f = consts.tile([P, NBLK, 4], f32)
        for s in range(NBLK):
            nc.gpsimd.indirect_dma_start(
                out=adb1f[:, s, :], out_offset=None, in_=AD1[:],
                in_offset=IOA(ap=blkit[:, s:s + 1], axis=0))
        nc.vector.tensor_copy(out=adbh1[:], in_=adb1f[:])

        # ---------------- generic edge phase (one slot per iteration) -------
        gq = [0]

        def edge_layer(LM, TBLa, TBLb, width, nfeat, as_f32col, adbh, nheads,
                       drelall, drelbx, idxall, ps_width, slot_epilogue):
            meta = LM["meta"]
            GW = LM["GW"]
            assert GW <= GWMAX
            fw = nfeat + nheads
            hd = nfeat // nheads
            for s in range(NBLK):
                c0, kb, ka = meta[s]
                nch = kb + ka
                drelb_t = ipool.tile([P, GW * P], u8, tag="drelb",
                                     name="drelb_t")
                nc.sync.dma_start(out=drelb_t[:, :nch * P],
                                  in_=drelbx[:, c0 * P:(c0 + nch) * P])
                grow = gpool.tile([P, GW, width], bf16, tag="grow", name="grow")
                # <=8 chunks (1024 idxs) per gather op (ucode limit). On HW,
                # round-robin the 4 SWDGE queues (sem-lane/queue mismatches
                # are tolerated); the strict full-exec sim needs queue 0.
                for tbl, lo, cnt in ((TBLb, 0, kb), (TBLa, kb, ka)):
                    j = 0
                    while j < cnt:
                        nj = min(8, cnt - j)
                        cj = c0 + lo + j
                        nc.gpsimd.dma_gather(
                            grow[:, lo + j:lo + j + nj, :], tbl[:],
                            idxall[:, cj * 8:(cj + nj) * 8], nj * P, nj * P,
                            width, queue_num=0 if _SIM_Q0 else gq[0] % 4)
                        gq[0] += 1
                        j += nj
                # S8: [e_part, chunk, d] one-hot (bf16 all the way)
                S8 = spool.tile([P, GW * P], bf16, tag="s8", name="s8")
                nc.vector.tensor_tensor(
                    out=_ap_view(S8[:], 0, [[P, nch], [1, P]]),
                    in0=_ap_view(drelall[:], c0, [[1, nch], [0, P]]),
                    in1=_ap_view(iotarow[:], 0, [[P, nch], [1, P]]),
                    op=OP.is_equal)
                # S_T: [d_part, chunk*P + e] one-hot from u8 drel broadcast;
                # both operands read contiguously.
                st8 = spool.tile([P, GW * P], bf16, tag="st8", name="st8")
                nc.vector.tensor_tensor(
                    out=st8[:, :nch * P], in0=drelb_t[:, :nch * P],
                    in1=iotacol[:, :nch * P], op=OP.is_equal)
                # alpha_dst expansion: adp[e, ch*nheads+h]
                ps = pse.tile([P, 200], f32, tag="ps", name="ps")
                adp = psad.tile([P, GW * nheads], f32, tag="adp", name="adp")
                for jj in range(nch):
                    nc.tensor.matmul(
                        out=adp[:, jj * nheads:(jj + 1) * nheads],
                        lhsT=st8[:, jj * P:(jj + 1) * P],
                        rhs=adbh[:, s, :nheads],
                        start=True, stop=True)
                # logits -> exp -> weighted features
                growf = grow[:].bitcast(f32)
                t8 = epool.tile([P, GW * nheads], f32, tag="t8", name="t8")
                nc.vector.tensor_tensor(
                    out=_ap_view(t8[:], 0, [[nheads, nch], [1, nheads]]),
                    in0=_ap_view(growf, as_f32col,
                                 [[width // 2, nch], [1, nheads]]),
                    in1=_ap_view(adp[:], 0, [[nheads, nch], [1, nheads]]),
                    op=OP.add)
                # exp(lrelu(t)) == max(exp(t), exp(SLOPE*t)) exactly
                e2 = epool.tile([P, GW * nheads], f32, tag="e2", name="e2")
                nc.scalar.activation(out=e2[:, :nch * nheads],
                                     in_=t8[:, :nch * nheads],
                                     func=AT.Exp, scale=SLOPE)
                F8 = fpool.tile([P, GW * fw], bf16, tag="f8", name="f8")
                nc.scalar.activation(
                    out=_ap_view(F8[:], nfeat, [[fw, nch], [1, nheads]]),
                    in_=_ap_view(t8[:], 0, [[nheads, nch], [1, nheads]]),
                    func=AT.Exp)
                nc.vector.tensor_tensor(
                    out=_ap_view(F8[:], nfeat, [[fw, nch], [1, nheads]]),
                    in0=_ap_view(F8[:], nfeat, [[fw, nch], [1, nheads]]),
                    in1=_ap_view(e2[:], 0, [[nheads, nch], [1, nheads]]),
                    op=OP.max)
                nc.vector.tensor_tensor(
                    out=_ap_view(F8[:], 0, [[fw, nch], [hd, nheads], [1, hd]]),
                    in0=_ap_view(grow[:], 0,
                                 [[width, nch], [hd, nheads], [1, hd]]),
                    in1=_ap_view(F8[:], nfeat,
                                 [[fw, nch], [1, nheads], [0, hd]]),
                    op=OP.mult)
                for jj in range(nch):
                    nc.tensor.matmul(
                        out=ps[:, :ps_width],
                        lhsT=S8[:, jj * P:(jj + 1) * P],
                        rhs=F8[:, jj * fw:jj * fw + ps_width],
                        start=(jj == 0), stop=(jj == nch - 1))
                slot_epilogue(s, ps)

        # L1 epilogue: h -> transpose -> G2 rows + ad2 (SBUF)
        def epi1(s, ps):
            rc = epool.tile([P, H], f32, tag="rc", name="rc")
            nc.vector.tensor_scalar_add(out=rc[:], in0=ps[:, F1:F1 + H],
                                        scalar1=EPS)
            rc2 = epool.tile([P, H], f32, tag="rc2", name="rc2")
            nc.vector.reciprocal(out=rc2[:], in_=rc[:])
            hm = epool.tile([P, F1], f32, tag="hm", name="hm")
            nc.vector.tensor_tensor(
                out=_ap_view(hm[:], 0, [[HID, H], [1, HID]]),
                in0=_ap_view(ps[:, :F1], 0, [[HID, H], [1, HID]]),
                in1=_ap_view(rc2[:], 0, [[1, H], [0, HID]]),
                op=OP.mult)
            hb = epool.tile([P, F1], f32, tag="hb", name="hb")
            nc.vector.tensor_tensor(out=hb[:], in0=hm[:], in1=b1t[:], op=OP.add)
            hr = epool.tile([P, F1], f32, tag="hr", name="hr")
            nc.scalar.activation(out=hr[:], in_=hb[:], func=AT.Relu)
            pt = pst.tile([P, 256], f32, tag="tr", name="pt")
            nc.tensor.transpose(out=pt[:, 0:P], in_=hr[:, :P], identity=ident[:])
            nc.tensor.transpose(out=pt[:F1 - P, P:256], in_=hr[:, P:F1],
                                identity=ident[:])
            ht1 = epool.tile([P, P], f32, tag="ht1", name="ht1")
            nc.vector.tensor_copy(out=ht1[:], in_=pt[:, 0:P])
            ht2 = epool.tile([F1 - P, P], f32, tag="ht2", name="ht2")
            nc.vector.tensor_copy(out=ht2[:], in_=pt[:F1 - P, P:256])
            pg = pge.tile([P, 200], f32, tag="pg", name="pg")
            nc.tensor.matmul(out=pg[:, :66], lhsT=ht1[:], rhs=rhs2_lo[:],
                             start=True, stop=False)
            nc.tensor.matmul(out=pg[:, :66], lhsT=ht2[:], rhs=rhs2_hi[:],
                             start=False, stop=True)
            g2 = epool.tile([P, G2W], bf16, tag="g2", name="g2")
            nc.vector.tensor_copy(out=g2[:, :F2], in_=pg[:, :F2])
            g2f = g2[:].bitcast(f32)
            nc.vector.memset(g2f[:, 33:64], 0.0)
            nc.vector.tensor_copy(out=g2f[:, 32:33], in_=pg[:, F2:F2 + 1])
            nc.vector.tensor_copy(out=adb2_sb[:, s, :1], in_=pg[:, F2 + 1:F2 + 2])
            nc.sync.dma_start(out=G2L[s * P:(s + 1) * P, :], in_=g2[:])

        edge_layer(L1, G1a, G1b, G1W, F1, 96, adbh1, H,
                   drelall1, DRELB1, idxall1, F1 + H, epi1)

        # ---------------- AllGather (G2 only) ----------------
        nc.gpsimd.collective_compute(
            "AllGather", mybir.AluOpType.bypass,
            replica_groups=[list(range(NCORES))],
            ins=[G2L.ap().opt()], outs=[G2F.ap().opt()])

        # ---------------- layer 2 ----------------
        def epi2(s, ps):
            rc = epool.tile([P, 1], f32, tag="rcB", name="rcB")
            nc.vector.tensor_scalar_add(out=rc[:], in0=ps[:, F2:F2 + 1],
                                        scalar1=EPS)
            rc2 = epool.tile([P, 1], f32, tag="rcB2", name="rcB2")
            nc.vector.reciprocal(out=rc2[:], in_=rc[:])
            om = epool.tile([P, F2], f32, tag="om", name="om")
            nc.vector.tensor_tensor(out=om[:], in0=ps[:, :F2],
                                    in1=rc2[:].to_broadcast([P, F2]),
                                    op=OP.mult)
            ob = epool.tile([P, F2], f32, tag="ob", name="ob")
            nc.vector.tensor_tensor(out=ob[:], in0=om[:], in1=b2t[:], op=OP.add)
            orl = epool.tile([P, F2], f32, tag="orl", name="orl")
            nc.scalar.activation(out=orl[:], in_=ob[:], func=AT.Relu)
            nc.sync.dma_start(out=OUT[s * P:(s + 1) * P, :], in_=orl[:])

        # G2F views for the two index halves (offsets stay < 2^24 bytes)
        g2fa = G2F[0:HALF, :]
        g2fb = G2F[HALF:NROWC * NCORES, :]
        edge_layer(L2, g2fa, g2fb, G2W, F2, 32, adb2_sb, 1,
                   drelall2, DRELB2, idxall2, F2 + 1, epi2)

    nc.compile()
    return nc


def _get_compiled(key, layers):
    if key not in _compiled:
        _compiled[key] = _build(layers[0], layers[1])
    return _compiled[key]


def run(inputs, **runkw):
    from concourse import bass_utils

    key, layers, shared, percore = _host_prep(inputs)
    nc = _get_compiled(key, layers)
    in_maps = []
    for c in range(NCORES):
        m = dict(shared)
        m.update(percore[c])
        in_maps.append(m)
    res = bass_utils.run_bass_kernel_spmd(
        nc, in_maps, core_ids=list(range(NCORES)), **runkw)
    return res


def assemble(results):
    out = np.empty((N, F2), dtype=np.float32)
    for c in range(NCORES):
        out[c * NPC:(c + 1) * NPC] = results[c]["out"][:NPC]
    return out


def kernel(**inputs):
    res = run(inputs)
    return assemble(res.results)


# revision 26
# speedup vs baseline: 2.5866x; 1.5603x over previous
"""GAT 2-layer kernel for Trainium2, 8 NeuronCores (SPMD, dst-sharded).

Strategy (v5):
  - Tile-aligned destination sharding: core c owns nodes [c*6272,(c+1)*6272)
    (6272 = 49*128; core 7 short). Edges bucketed into per-128-dst-node "slots"
    (49/core), padded to 128-edge chunks.
  - Node space is split into two pieces per core (A: first 25 tiles, B: last 24)
    so every gather table stays under 32768 rows (int16 gather indices) AND the
    G2 AllGather can run as two collectives, the first overlapping L1's tail.
    L1 tables G1A/G1B and L2 tables G2FA/G2FB use the SAME row ids -> one shared
    chunk structure, index array and drel-broadcast array for both layers.
  - Stage A (replicated, batched x4): per 128-node tile one matmul computes
    [x@W1 (192) | alpha_src (3) | alpha_dst (3)]; xw+as -> bf16 gather rows
    (512B); alpha_dst accumulated straight into a slot-layout SBUF tile via a
    per-core {0,1} mask input (no DRAM roundtrip, no indirect DMA).
  - Edge phase per layer, one slot per iteration: <=8-chunk dma_gathers pull the
    slot's source rows; one-hot S (DVE is_equal vs iota consts) segment-reduces
    exp-weighted features AND softmax denominators per-slot in PSUM; alpha_dst
    expanded edge-wise via host bf16 drel-broadcast + DVE is_equal (2x mode) ->
    per-chunk S_T matmul vs the slot's alpha_dst block (SBUF-resident).
  - Per-slot epilogue: h = relu(sum/(denom+eps) + bias1); PE-transpose h, emit
    G2 rows [h@W2 (64) bf16 | as2 f32] to G2LA/G2LB and ad2 to SBUF; layer 2
    repeats the edge phase (1 head) against G2FA/G2FB.
"""
import sys

sys.path.insert(0, "/opt/trn_rl_repo")
import numpy as np
import ml_dtypes

N = 50000
D = 128
HID = 64
H = 3
F1 = 192
F2 = 64
NCORES = 8
P = 128
NBLK = 49                  # slots per core
NPC = NBLK * P             # 6272 padded nodes per core
NT = (N + P - 1) // P      # 391 stage-A node tiles
NROW1 = NT * P             # 50048 padded node rows
PA_T = 25                  # piece-A tiles (slots) per core
PB_T = NBLK - PA_T         # 24
RA = PA_T * P              # 3200 piece-A rows per core
RB = PB_T * P              # 3072
NRA = NCORES * RA          # 25600 (< 32768: int16-safe)
NRB = NCORES * RB          # 24576
G1W = 256                  # bf16 cols: xw(192) | as f32 x3 (bf16 192:198) | pad
G2W = 128                  # bf16 cols: xw2(64) | as2 f32 (bf16 64:66) | pad
SLOPE = 0.2
EPS = 1e-16
BA = 4                     # stage-A tiles per DMA batch
GWMAX = 24                 # max chunks per slot supported by iota consts

_compiled = {}


def _rowid(node):
    """Piece-split row id for a node (same for G1 and G2 tables)."""
    c = node // NPC
    rel = node % NPC
    a = rel < RA
    return np.where(a, c * RA + rel, c * RB + rel - RA), a


def _host_prep(inputs):
    x = np.asarray(inputs["x"], dtype=np.float32)
    ei = np.asarray(inputs["edge_index"])
    W1 = np.asarray(inputs["W1"], dtype=np.float32)
    as1 = np.asarray(inputs["att_src1"], dtype=np.float32)
    ad1 = np.asarray(inputs["att_dst1"], dtype=np.float32)
    b1 = np.asarray(inputs["bias1"], dtype=np.float32)
    W2 = np.asarray(inputs["W2"], dtype=np.float32)
    as2 = np.asarray(inputs["att_src2"], dtype=np.float32)
    ad2 = np.asarray(inputs["att_dst2"], dtype=np.float32)
    b2 = np.asarray(inputs["bias2"], dtype=np.float32)

    loops = np.arange(N, dtype=np.int64)
    src = np.concatenate([ei[0].astype(np.int64), loops])
    dst = np.concatenate([ei[1].astype(np.int64), loops])
    order = np.argsort(dst, kind="stable")
    src = src[order]
    dst = dst[order]
    srow, sa = _rowid(src)

    # per-slot chunk counts (max over cores so the program is uniform);
    # A-half chunks first within a slot.
    core = dst // NPC
    rel = dst % NPC
    slot = rel // P
    counts = np.zeros((NCORES, NBLK, 2), dtype=np.int64)
    np.add.at(counts, (core, slot, (~sa).astype(np.int64)), 1)
    Ka = np.ceil(counts[:, :, 0] / P).astype(np.int64).max(axis=0)
    Kb = np.ceil(counts[:, :, 1] / P).astype(np.int64).max(axis=0)
    Ktot = Ka + Kb
    NCH = int(Ktot.sum())
    GW = int(Ktot.max())
    meta = []              # per slot: (chunk_base, Ka, Kb)
    cb = 0
    for s in range(NBLK):
        meta.append((cb, int(Ka[s]), int(Kb[s])))
        cb += int(Ktot[s])

    EPAD = NCH * P
    SROWK = np.zeros((NCORES, EPAD), dtype=np.int64)
    DREL = np.full((NCORES, EPAD), 255.0, dtype=np.float32)
    for c in range(NCORES):
        base_node = c * NPC
        cb = 0
        for s in range(NBLK):
            blo = base_node + s * P
            bhi = blo + P
            lo = np.searchsorted(dst, blo, side="left")
            hi = np.searchsorted(dst, bhi, side="left")
            sr = srow[lo:hi]
            am = sa[lo:hi]
            dr = (dst[lo:hi] - blo).astype(np.float32)
            for which, KK in ((am, Ka[s]), (~am, Kb[s])):
                cnt = int(which.sum())
                pos = cb * P
                SROWK[c, pos:pos + cnt] = sr[which]
                SROWK[c, pos + cnt:(cb + int(KK)) * P] = 0
                DREL[c, pos:pos + cnt] = dr[which]
                cb += int(KK)
    DREL_t = np.ascontiguousarray(
        DREL.reshape(NCORES, NCH, P).transpose(0, 2, 1)
    ).astype(ml_dtypes.bfloat16)                       # [C, P, NCH]
    DRELB = np.ascontiguousarray(np.broadcast_to(
        DREL.astype(ml_dtypes.bfloat16).reshape(NCORES, 1, EPAD),
        (NCORES, P, EPAD)).transpose(1, 0, 2)).transpose(1, 0, 2)
    # wrapped int16 indices, per chunk 8 cols: [P, NCH*8]
    IDXW = np.zeros((NCORES, P, NCH * 8), dtype=np.int16)
    for c in range(NCORES):
        iv = SROWK[c].reshape(-1, 16)                  # [NCH*8, 16]
        w = iv.reshape(NCH, 8, 16).transpose(0, 2, 1).astype(np.int16)
        # per chunk: [16, 8] tiled to [128, 8]
        for ch in range(NCH):
            IDXW[c, :, ch * 8:(ch + 1) * 8] = np.tile(w[ch], (8, 1))
    L = dict(NCH=NCH, GW=GW, meta=meta, Ktot=[int(k) for k in Ktot])

    # per-core tile-ownership mask for alpha_dst slot accumulation
    MASKT = np.zeros((NCORES, P, NT), dtype=np.float32)
    for c in range(NCORES):
        t0 = c * NBLK
        t1 = min(NT, (c + 1) * NBLK)
        MASKT[c, :, t0:t1] = 1.0

    xT = np.zeros((D, NROW1), dtype=np.float32)
    xT[:, :N] = x.T
    A1 = np.zeros((F1, 6), dtype=np.float32)
    for h in range(H):
        A1[h * HID:(h + 1) * HID, h] = as1[h]
        A1[h * HID:(h + 1) * HID, 3 + h] = ad1[h]
    A2 = np.stack([as2[0], ad2[0]], axis=1).astype(np.float32)

    shared = {
        "xT": xT,
        "W1": np.ascontiguousarray(W1),
        "W1T": np.ascontiguousarray(W1.T),
        "A1": A1,
        "W2": np.ascontiguousarray(W2),
        "W2T": np.ascontiguousarray(W2.T),
        "A2": A2,
        "B1": np.ascontiguousarray(np.broadcast_to(b1, (P, F1))),
        "B2": np.ascontiguousarray(np.broadcast_to(b2, (P, F2))),
        "IOTAROW": np.ascontiguousarray(np.broadcast_to(
            np.tile(np.arange(P, dtype=ml_dtypes.bfloat16), GWMAX),
            (P, GWMAX * P))),
        "IOTACOL": np.ascontiguousarray(np.broadcast_to(
            np.arange(P, dtype=ml_dtypes.bfloat16).reshape(P, 1),
            (P, GWMAX * P))),
    }
    percore = []
    for c in range(NCORES):
        percore.append({
            "DREL1": DREL_t[c], "DRELB1": DRELB[c], "IDXW1": IDXW[c],
            "MASKT": MASKT[c],
        })
    key = tuple(L["Ktot"])
    return key, L, shared, percore


def _ap_view(ap, extra_offset, free_dims):
    import concourse.bass as bass

    return bass.AP(
        tensor=ap.tensor, offset=ap.offset + extra_offset,
        ap=[list(ap.ap[0])] + [list(d) for d in free_dims],
    )


def _dram_ap(dt_handle, offset, dims):
    import concourse.bass as bass

    ap = dt_handle.ap()
    return bass.AP(tensor=ap.tensor, offset=offset,
                   ap=[list(d) for d in dims])


def _build(L):
    import os
    global _SIM_Q0
    _SIM_Q0 = bool(int(os.environ.get("BASS_SIM_Q0", "0")))
    import concourse.bass as bass
    import concourse.bacc as bacc
    import concourse.tile as tile
    from concourse import mybir
    from concourse.masks import make_identity
    from concourse.library_config import mlp
    from contextlib import ExitStack

    f32 = mybir.dt.float32
    bf16 = mybir.dt.bfloat16
    i16 = mybir.dt.int16
    AT = mybir.ActivationFunctionType
    OP = mybir.AluOpType

    nc = bacc.Bacc("TRN2", target_bir_lowering=False, debug=False,
                   num_devices=NCORES, num_swdge_queues=4)

    NCH = L["NCH"]
    GW = L["GW"]
    assert GW <= GWMAX

    xT = nc.dram_tensor("xT", [D, NROW1], f32, kind="ExternalInput")
    W1 = nc.dram_tensor("W1", [D, F1], f32, kind="ExternalInput")
    W1T = nc.dram_tensor("W1T", [F1, D], f32, kind="ExternalInput")
    A1 = nc.dram_tensor("A1", [F1, 6], f32, kind="ExternalInput")
    W2 = nc.dram_tensor("W2", [F1, F2], f32, kind="ExternalInput")
    W2T = nc.dram_tensor("W2T", [F2, F1], f32, kind="ExternalInput")
    A2 = nc.dram_tensor("A2", [F2, 2], f32, kind="ExternalInput")
    B1 = nc.dram_tensor("B1", [P, F1], f32, kind="ExternalInput")
    B2 = nc.dram_tensor("B2", [P, F2], f32, kind="ExternalInput")
    IOTAROW = nc.dram_tensor("IOTAROW", [P, GWMAX * P], bf16,
                             kind="ExternalInput")
    IOTACOL = nc.dram_tensor("IOTACOL", [P, GWMAX * P], bf16,
                             kind="ExternalInput")
    DREL1 = nc.dram_tensor("DREL1", [P, NCH], bf16, kind="ExternalInput")
    DRELB1 = nc.dram_tensor("DRELB1", [P, NCH * P], bf16, kind="ExternalInput")
    IDXW1 = nc.dram_tensor("IDXW1", [P, NCH * 8], i16, kind="ExternalInput")
    MASKT = nc.dram_tensor("MASKT", [P, NT], f32, kind="ExternalInput")
    OUT = nc.dram_tensor("out", [NPC, F2], f32, kind="ExternalOutput")

    G1A = nc.dram_tensor("G1A", [NRA, G1W], bf16, kind="Internal")
    G1B = nc.dram_tensor("G1B", [NRB, G1W], bf16, kind="Internal")
    G2LA = nc.dram_tensor("G2LA", [RA, G2W], bf16, kind="Internal")
    G2LB = nc.dram_tensor("G2LB", [RB, G2W], bf16, kind="Internal")
    G2FA = nc.dram_tensor("G2FA", [NRA, G2W], bf16,
                          addr_space="Shared", kind="Internal")
    G2FB = nc.dram_tensor("G2FB", [NRB, G2W], bf16,
                          addr_space="Shared", kind="Internal")

    with tile.TileContext(nc) as tc, ExitStack() as ctx:
        consts = ctx.enter_context(tc.tile_pool(name="consts", bufs=1))
        sbA = ctx.enter_context(tc.tile_pool(name="sbA", bufs=4))
        pas = ctx.enter_context(tc.tile_pool(name="pas", bufs=3, space="PSUM"))
        pse = ctx.enter_context(tc.tile_pool(name="pse", bufs=2, space="PSUM"))
        psad = ctx.enter_context(tc.tile_pool(name="psad", bufs=1, space="PSUM"))
        pst = ctx.enter_context(tc.tile_pool(name="pst", bufs=1, space="PSUM"))
        pge = ctx.enter_context(tc.tile_pool(name="pge", bufs=1, space="PSUM"))
        gpool = ctx.enter_context(tc.tile_pool(name="gpool", bufs=6))
        fpool = ctx.enter_context(tc.tile_pool(name="fpool", bufs=4))
        spool = ctx.enter_context(tc.tile_pool(name="spool", bufs=5))
        ipool = ctx.enter_context(tc.tile_pool(name="ipool", bufs=6))
        epool = ctx.enter_context(tc.tile_pool(name="epool", bufs=4))

        nc.gpsimd.load_library(mlp)

        # ---------------- constants / preloads ----------------
        iotarow = consts.tile([P, GWMAX * P], bf16)
        nc.sync.dma_start(out=iotarow[:], in_=IOTAROW[:])
        iotacol = consts.tile([P, GWMAX * P], bf16)
        nc.sync.dma_start(out=iotacol[:], in_=IOTACOL[:])
        b1t = consts.tile([P, F1], f32)
        nc.sync.dma_start(out=b1t[:], in_=B1[:])
        b2t = consts.tile([P, F2], f32)
        nc.sync.dma_start(out=b2t[:], in_=B2[:])
        ident = consts.tile([P, P], f32)
        make_identity(nc, ident[:])

        idxall = consts.tile([P, NCH * 8], i16)
        nc.sync.dma_start(out=idxall[:], in_=IDXW1[:])
        drelall = consts.tile([P, NCH], bf16)
        nc.sync.dma_start(out=drelall[:], in_=DREL1[:])
        maskt = consts.tile([P, NT], f32)
        nc.sync.dma_start(out=maskt[:], in_=MASKT[:])

        rhs1 = consts.tile([P, 198], f32)
        nc.sync.dma_start(out=rhs1[:, :F1], in_=W1[:])
        w1t_a = consts.tile([P, D], f32)
        nc.sync.dma_start(out=w1t_a[:], in_=W1T[0:P, :])
        w1t_b = consts.tile([F1 - P, D], f32)
        nc.sync.dma_start(out=w1t_b[:], in_=W1T[P:F1, :])
        a1_a = consts.tile([P, 6], f32)
        nc.sync.dma_start(out=a1_a[:], in_=A1[0:P, :])
        a1_b = consts.tile([F1 - P, 6], f32)
        nc.sync.dma_start(out=a1_b[:], in_=A1[P:F1, :])
        pu = pge.tile([P, 200], f32, tag="pg")
        nc.tensor.matmul(out=pu[:, :6], lhsT=w1t_a[:], rhs=a1_a[:],
                         start=True, stop=False)
        nc.tensor.matmul(out=pu[:, :6], lhsT=w1t_b[:], rhs=a1_b[:],
                         start=False, stop=True)
        nc.vector.tensor_copy(out=rhs1[:, F1:F1 + 6], in_=pu[:, :6])

        w2t = consts.tile([F2, F1], f32)
        nc.sync.dma_start(out=w2t[:], in_=W2T[:])
        a2t = consts.tile([F2, 2], f32)
        nc.sync.dma_start(out=a2t[:], in_=A2[:])
        rhs2_lo = consts.tile([P, 66], f32)
        nc.sync.dma_start(out=rhs2_lo[:, :F2], in_=W2[0:P, :])
        rhs2_hi = consts.tile([F1 - P, 66], f32)
        nc.sync.dma_start(out=rhs2_hi[:, :F2], in_=W2[P:F1, :])
        pu2 = pst.tile([P, 256], f32, tag="tr")
        nc.tensor.matmul(out=pu2[:, :2], lhsT=w2t[:, 0:P], rhs=a2t[:],
                         start=True, stop=True)
        nc.vector.tensor_copy(out=rhs2_lo[:, F2:F2 + 2], in_=pu2[:, :2])
        pu2b = pst.tile([P, 256], f32, tag="tr")
        nc.tensor.matmul(out=pu2b[:F1 - P, 4:6], lhsT=w2t[:, P:F1], rhs=a2t[:],
                         start=True, stop=True)
        nc.vector.tensor_copy(out=rhs2_hi[:, F2:F2 + 2], in_=pu2b[:F1 - P, 4:6])

        # alpha_dst-block tables (slot-major, partition = node-within-slot)
        adsb = consts.tile([P, NBLK, 4], f32)
        nc.vector.memset(adsb[:], 0.0)
        adbh1 = consts.tile([P, NBLK, 4], bf16)
        adb2_sb = consts.tile([P, NBLK, 1], bf16)

        # ---------------- stage A (piece-A tiles first), batched -------------
        def stage_a_group(t0, nb):
            c = t0 // NBLK
            i0 = t0 - c * NBLK
            xt = sbA.tile([P, BA * P], f32, tag="xt")
            nc.sync.dma_start(out=xt[:, :nb * P],
                              in_=xT[:, t0 * P:(t0 + nb) * P])
            gbf = sbA.tile([P, BA, G1W], bf16, tag="gbf")
            gf32 = gbf[:].bitcast(f32)
            nc.vector.memset(gf32[:, :, 99:128], 0.0)
            for b in range(nb):
                t = t0 + b
                pa = pas.tile([P, 200], f32, tag="pa")
                nc.tensor.matmul(out=pa[:, :198], lhsT=xt[:, b * P:(b + 1) * P],
                                 rhs=rhs1[:], start=True, stop=True)
                nc.scalar.activation(out=gbf[:, b, :F1], in_=pa[:, :F1],
                                     func=AT.Copy)
                nc.vector.tensor_copy(out=gf32[:, b, 96:99],
                                      in_=pa[:, F1:F1 + 3])
                adm = sbA.tile([P, 4], f32, tag="adm")
                nc.vector.tensor_scalar(
                    out=adm[:, :3], in0=pa[:, F1 + 3:F1 + 6],
                    scalar1=maskt[:, t:t + 1], scalar2=None, op0=OP.mult)
                col = t % NBLK
                nc.vector.tensor_tensor(
                    out=adsb[:, col, :3], in0=adsb[:, col, :3],
                    in1=adm[:, :3], op=OP.add)
            if i0 < PA_T:
                dst_ap = _dram_ap(G1A, (c * RA + i0 * P) * G1W,
                                  [[G1W, P], [P * G1W, nb], [1, G1W]])
            else:
                dst_ap = _dram_ap(G1B, (c * RB + (i0 - PA_T) * P) * G1W,
                                  [[G1W, P], [P * G1W, nb], [1, G1W]])
            nc.scalar.dma_start(out=dst_ap, in_=gbf[:, :nb, :])

        # piece-A tiles (all cores) first, then piece-B
        for piece in (0, 1):
            for c in range(NCORES):
                lo = c * NBLK + (0 if piece == 0 else PA_T)
                hi = min(c * NBLK + (PA_T if piece == 0 else NBLK),
                         NT)
                t = lo
                while t < hi:
                    nb = min(BA, hi - t)
                    stage_a_group(t, nb)
                    t += nb

        nc.vector.tensor_copy(out=adbh1[:], in_=adsb[:])

        # ---------------- generic edge phase (one slot per iteration) -------
        gq = [0]

        def edge_layer(TBLa, TBLb, width, nfeat, as_f32col, adbh, nheads,
                       ps_width, slot_epilogue, after_slot=None):
            meta = L["meta"]
            fw = nfeat + nheads
            hd = nfeat // nheads
            for s in range(NBLK):
                c0, ka, kb = meta[s]
                nch = ka + kb
                drelb_t = ipool.tile([P, GW * P], bf16, tag="drelb",
                                     name="drelb_t")
                nc.sync.dma_start(out=drelb_t[:, :nch * P],
                                  in_=DRELB1[:, c0 * P:(c0 + nch) * P])
                grow = gpool.tile([P, GW, width], bf16, tag="grow", name="grow")
                for tbl, lo, cnt in ((TBLa, 0, ka), (TBLb, ka, kb)):
                    j = 0
                    while j < cnt:
                        nj = min(8, cnt - j)
                        cj = c0 + lo + j
                        nc.gpsimd.dma_gather(
                            grow[:, lo + j:lo + j + nj, :], tbl[:],
                            idxall[:, cj * 8:(cj + nj) * 8], nj * P, nj * P,
                            width, queue_num=0 if _SIM_Q0 else gq[0] % 4)
                        gq[0] += 1
                        j += nj
                # S8: [e_part, chunk, d] one-hot (bf16)
                S8 = spool.tile([P, GW * P], bf16, tag="s8", name="s8")
                nc.vector.tensor_tensor(
                    out=_ap_view(S8[:], 0, [[P, nch], [1, P]]),
                    in0=_ap_view(drelall[:], c0, [[1, nch], [0, P]]),
                    in1=_ap_view(iotarow[:], 0, [[P, nch], [1, P]]),
                    op=OP.is_equal)
                # S_T: [d_part, chunk*P + e] one-hot; all-bf16 packed (2x DVE)
                st8 = spool.tile([P, GW * P], bf16, tag="st8", name="st8")
                nc.vector.tensor_tensor(
                    out=st8[:, :nch * P], in0=drelb_t[:, :nch * P],
                    in1=iotacol[:, :nch * P], op=OP.is_equal)
                # alpha_dst expansion: adp[e, ch*nheads+h]
                ps = pse.tile([P, 200], f32, tag="ps", name="ps")
                adp = psad.tile([P, GWMAX * H], f32, tag="adp", name="adp")
                for jj in range(nch):
                    nc.tensor.matmul(
                        out=adp[:, jj * nheads:(jj + 1) * nheads],
                        lhsT=st8[:, jj * P:(jj + 1) * P],
                        rhs=adbh[:, s, :nheads],
                        start=True, stop=True)
                # logits -> exp -> weighted features
                growf = grow[:].bitcast(f32)
                t8 = epool.tile([P, GWMAX * H], f32, tag="t8", name="t8")
                nc.vector.tensor_tensor(
                    out=_ap_view(t8[:], 0, [[nheads, nch], [1, nheads]]),
                    in0=_ap_view(growf, as_f32col,
                                 [[width // 2, nch], [1, nheads]]),
                    in1=_ap_view(adp[:], 0, [[nheads, nch], [1, nheads]]),
                    op=OP.add)
                # exp(lrelu(t)) == max(exp(t), exp(SLOPE*t)) exactly
                e2 = epool.tile([P, GWMAX * H], f32, tag="e2", name="e2")
                nc.scalar.activation(out=e2[:, :nch * nheads],
                                     in_=t8[:, :nch * nheads],
                                     func=AT.Exp, scale=SLOPE)
                F8 = fpool.tile([P, GW * fw], bf16, tag="f8", name="f8")
                nc.scalar.activation(
                    out=_ap_view(F8[:], nfeat, [[fw, nch], [1, nheads]]),
                    in_=_ap_view(t8[:], 0, [[nheads, nch], [1, nheads]]),
                    func=AT.Exp)
                nc.vector.tensor_tensor(
                    out=_ap_view(F8[:], nfeat, [[fw, nch], [1, nheads]]),
                    in0=_ap_view(F8[:], nfeat, [[fw, nch], [1, nheads]]),
                    in1=_ap_view(e2[:], 0, [[nheads, nch], [1, nheads]]),
                    op=OP.max)
                nc.vector.tensor_tensor(
                    out=_ap_view(F8[:], 0, [[fw, nch], [hd, nheads], [1, hd]]),
                    in0=_ap_view(grow[:], 0,
                                 [[width, nch], [hd, nheads], [1, hd]]),
                    in1=_ap_view(F8[:], nfeat,
                                 [[fw, nch], [1, nheads], [0, hd]]),
                    op=OP.mult)
                for jj in range(nch):
                    nc.tensor.matmul(
                        out=ps[:, :ps_width],
                        lhsT=S8[:, jj * P:(jj + 1) * P],
                        rhs=F8[:, jj * fw:jj * fw + ps_width],
                        start=(jj == 0), stop=(jj == nch - 1))
                slot_epilogue(s, ps)
                if after_slot is not None:
                    after_slot(s)

        # L1 epilogue: h -> transpose -> G2 rows + ad2 (SBUF)
        def epi1(s, ps):
            rc = epool.tile([P, H], f32, tag="rc", name="rc")
            nc.vector.tensor_scalar_add(out=rc[:], in0=ps[:, F1:F1 + H],
                                        scalar1=EPS)
            rc2 = epool.tile([P, H], f32, tag="rc2", name="rc2")
            nc.vector.reciprocal(out=rc2[:], in_=rc[:])
            hm = epool.tile([P, F1], f32, tag="hm", name="hm")
            nc.vector.tensor_tensor(
                out=_ap_view(hm[:], 0, [[HID, H], [1, HID]]),
                in0=_ap_view(ps[:, :F1], 0, [[HID, H], [1, HID]]),
                in1=_ap_view(rc2[:], 0, [[1, H], [0, HID]]),
                op=OP.mult)
            hb = epool.tile([P, F1], f32, tag="hb", name="hb")
            nc.vector.tensor_tensor(out=hb[:], in0=hm[:], in1=b1t[:], op=OP.add)
            hr = epool.tile([P, F1], f32, tag="hr", name="hr")
            nc.scalar.activation(out=hr[:], in_=hb[:], func=AT.Relu)
            pt = pst.tile([P, 256], f32, tag="tr", name="pt")
            nc.tensor.transpose(out=pt[:, 0:P], in_=hr[:, :P], identity=ident[:])
            nc.tensor.transpose(out=pt[:F1 - P, P:256], in_=hr[:, P:F1],
                                identity=ident[:])
            ht1 = epool.tile([P, P], f32, tag="ht1", name="ht1")
            nc.vector.tensor_copy(out=ht1[:], in_=pt[:, 0:P])
            ht2 = epool.tile([F1 - P, P], f32, tag="ht2", name="ht2")
            nc.vector.tensor_copy(out=ht2[:], in_=pt[:F1 - P, P:256])
            pg = pge.tile([P, 200], f32, tag="pg", name="pg")
            nc.tensor.matmul(out=pg[:, :66], lhsT=ht1[:], rhs=rhs2_lo[:],
                             start=True, stop=False)
            nc.tensor.matmul(out=pg[:, :66], lhsT=ht2[:], rhs=rhs2_hi[:],
                             start=False, stop=True)
            g2 = epool.tile([P, G2W], bf16, tag="g2", name="g2")
            nc.vector.tensor_copy(out=g2[:, :F2], in_=pg[:, :F2])
            g2f = g2[:].bitcast(f32)
            nc.vector.memset(g2f[:, 33:64], 0.0)
            nc.vector.tensor_copy(out=g2f[:, 32:33], in_=pg[:, F2:F2 + 1])
            nc.vector.tensor_copy(out=adb2_sb[:, s, :1], in_=pg[:, F2 + 1:F2 + 2])
            if s < PA_T:
                nc.sync.dma_start(out=G2LA[s * P:(s + 1) * P, :], in_=g2[:])
            else:
                sb = s - PA_T
                nc.sync.dma_start(out=G2LB[sb * P:(sb + 1) * P, :], in_=g2[:])

        def after1(s):
            from concourse import mybir as mb
            if s == PA_T - 1:
                nc.gpsimd.collective_compute(
                    "AllGather", mb.AluOpType.bypass,
                    replica_groups=[list(range(NCORES))],
                    ins=[G2LA.ap().opt()], outs=[G2FA.ap().opt()])
            if s == NBLK - 1:
                nc.gpsimd.collective_compute(
                    "AllGather", mb.AluOpType.bypass,
                    replica_groups=[list(range(NCORES))],
                    ins=[G2LB.ap().opt()], outs=[G2FB.ap().opt()])

        edge_layer(G1A, G1B, G1W, F1, 96, adbh1, H, F1 + H, epi1,
                   after_slot=after1)

        # ---------------- layer 2 ----------------
        def epi2(s, ps):
            rc = epool.tile([P, 1], f32, tag="rcB", name="rcB")
            nc.vector.tensor_scalar_add(out=rc[:], in0=ps[:, F2:F2 + 1],
                                        scalar1=EPS)
            rc2 = epool.tile([P, 1], f32, tag="rcB2", name="rcB2")
            nc.vector.reciprocal(out=rc2[:], in_=rc[:])
            om = epool.tile([P, F2], f32, tag="om", name="om")
            nc.vector.tensor_tensor(out=om[:], in0=ps[:, :F2],
                                    in1=rc2[:].to_broadcast([P, F2]),
                                    op=OP.mult)
            ob = epool.tile([P, F2], f32, tag="ob", name="ob")
            nc.vector.tensor_tensor(out=ob[:], in0=om[:], in1=b2t[:], op=OP.add)
            orl = epool.tile([P, F2], f32, tag="orl", name="orl")
            nc.scalar.activation(out=orl[:], in_=ob[:], func=AT.Relu)
            nc.sync.dma_start(out=OUT[s * P:(s + 1) * P, :], in_=orl[:])

        edge_layer(G2FA, G2FB, G2W, F2, 32, adb2_sb, 1, F2 + 1, epi2)

    nc.compile()
    return nc


def _get_compiled(key, L):
    if key not in _compiled:
        _compiled[key] = _build(L)
    return _compiled[key]


def run(inputs, **runkw):
    from concourse import bass_utils

    key, L, shared, percore = _host_prep(inputs)
    nc = _get_compiled(key, L)
    in_maps = []
    for c in range(NCORES):
        m = dict(shared)
        m.update(percore[c])
        in_maps.append(m)
    res = bass_utils.run_bass_kernel_spmd(
        nc, in_maps, core_ids=list(range(NCORES)), **runkw)
    return res


def assemble(results):
    out = np.empty((N, F2), dtype=np.float32)
    for c in range(NCORES):
        lo = c * NPC
        valid = min(NPC, N - lo)
        out[lo:lo + valid] = results[c]["out"][:valid]
    return out


def kernel(**inputs):
    res = run(inputs)
    return assemble(res.results)


# revision 32
# speedup vs baseline: 2.6888x; 1.0395x over previous
"""GAT 2-layer kernel for Trainium2, 8 NeuronCores (SPMD, dst-sharded).

Strategy (v5):
  - Tile-aligned destination sharding: core c owns nodes [c*6272,(c+1)*6272)
    (6272 = 49*128; core 7 short). Edges bucketed into per-128-dst-node "slots"
    (49/core), padded to 128-edge chunks.
  - Node space is split into two pieces per core (A: first 25 tiles, B: last 24)
    so every gather table stays under 32768 rows (int16 gather indices) AND the
    G2 AllGather can run as two collectives, the first overlapping L1's tail.
    L1 tables G1A/G1B and L2 tables G2FA/G2FB use the SAME row ids -> one shared
    chunk structure, index array and drel-broadcast array for both layers.
  - Stage A (replicated, batched x4): per 128-node tile one matmul computes
    [x@W1 (192) | alpha_src (3) | alpha_dst (3)]; xw+as -> bf16 gather rows
    (512B); alpha_dst accumulated straight into a slot-layout SBUF tile via a
    per-core {0,1} mask input (no DRAM roundtrip, no indirect DMA).
  - Edge phase per layer, one slot per iteration: <=8-chunk dma_gathers pull the
    slot's source rows; one-hot S (DVE is_equal vs iota consts) segment-reduces
    exp-weighted features AND softmax denominators per-slot in PSUM; alpha_dst
    expanded edge-wise via host bf16 drel-broadcast + DVE is_equal (2x mode) ->
    per-chunk S_T matmul vs the slot's alpha_dst block (SBUF-resident).
  - Per-slot epilogue: h = relu(sum/(denom+eps) + bias1); PE-transpose h, emit
    G2 rows [h@W2 (64) bf16 | as2 f32] to G2LA/G2LB and ad2 to SBUF; layer 2
    repeats the edge phase (1 head) against G2FA/G2FB.
"""
import sys

sys.path.insert(0, "/opt/trn_rl_repo")
import numpy as np
import ml_dtypes

N = 50000
D = 128
HID = 64
H = 3
F1 = 192
F2 = 64
NCORES = 8
P = 128
NBLK = 49                  # slots per core
NPC = NBLK * P             # 6272 padded nodes per core
NT = (N + P - 1) // P      # 391 stage-A node tiles
NROW1 = NT * P             # 50048 padded node rows
PA_T = 32                  # piece-A tiles (slots) per core (8*32*128 = 32768
                           # rows: exactly the int16 gather-index limit)
PB_T = NBLK - PA_T         # 17
RA = PA_T * P              # 3200 piece-A rows per core
RB = PB_T * P              # 3072
NRA = NCORES * RA          # 25600 (< 32768: int16-safe)
NRB = NCORES * RB          # 24576
G1W = 256                  # bf16 cols: xw(192) | as f32 x3 (bf16 192:198) | pad
G2W = 128                  # bf16 cols: xw2(64) | as2 f32 (bf16 64:66) | pad
SLOPE = 0.2
EPS = 1e-16
BA = 4                     # stage-A tiles per DMA batch
GWMAX = 24                 # max chunks per slot supported by iota consts

_compiled = {}


def _rowid(node):
    """Piece-split row id for a node (same for G1 and G2 tables)."""
    c = node // NPC
    rel = node % NPC
    a = rel < RA
    return np.where(a, c * RA + rel, c * RB + rel - RA), a


def _host_prep(inputs):
    x = np.asarray(inputs["x"], dtype=np.float32)
    ei = np.asarray(inputs["edge_index"])
    W1 = np.asarray(inputs["W1"], dtype=np.float32)
    as1 = np.asarray(inputs["att_src1"], dtype=np.float32)
    ad1 = np.asarray(inputs["att_dst1"], dtype=np.float32)
    b1 = np.asarray(inputs["bias1"], dtype=np.float32)
    W2 = np.asarray(inputs["W2"], dtype=np.float32)
    as2 = np.asarray(inputs["att_src2"], dtype=np.float32)
    ad2 = np.asarray(inputs["att_dst2"], dtype=np.float32)
    b2 = np.asarray(inputs["bias2"], dtype=np.float32)

    loops = np.arange(N, dtype=np.int64)
    src = np.concatenate([ei[0].astype(np.int64), loops])
    dst = np.concatenate([ei[1].astype(np.int64), loops])
    order = np.argsort(dst, kind="stable")
    src = src[order]
    dst = dst[order]
    srow, sa = _rowid(src)

    # per-slot chunk counts (max over cores so the program is uniform);
    # A-half chunks first within a slot.
    core = dst // NPC
    rel = dst % NPC
    slot = rel // P
    counts = np.zeros((NCORES, NBLK, 2), dtype=np.int64)
    np.add.at(counts, (core, slot, (~sa).astype(np.int64)), 1)
    Ka = np.ceil(counts[:, :, 0] / P).astype(np.int64).max(axis=0)
    Kb = np.ceil(counts[:, :, 1] / P).astype(np.int64).max(axis=0)
    Ktot = Ka + Kb
    NCH = int(Ktot.sum())
    GW = int(Ktot.max())
    meta = []              # per slot: (chunk_base, Ka, Kb)
    cb = 0
    for s in range(NBLK):
        meta.append((cb, int(Ka[s]), int(Kb[s])))
        cb += int(Ktot[s])

    EPAD = NCH * P
    SROWK = np.zeros((NCORES, EPAD), dtype=np.int64)
    DREL = np.full((NCORES, EPAD), 255.0, dtype=np.float32)
    for c in range(NCORES):
        base_node = c * NPC
        cb = 0
        for s in range(NBLK):
            blo = base_node + s * P
            bhi = blo + P
            lo = np.searchsorted(dst, blo, side="left")
            hi = np.searchsorted(dst, bhi, side="left")
            sr = srow[lo:hi]
            am = sa[lo:hi]
            dr = (dst[lo:hi] - blo).astype(np.float32)
            for which, KK in ((am, Ka[s]), (~am, Kb[s])):
                cnt = int(which.sum())
                pos = cb * P
                SROWK[c, pos:pos + cnt] = sr[which]
                SROWK[c, pos + cnt:(cb + int(KK)) * P] = 0
                DREL[c, pos:pos + cnt] = dr[which]
                cb += int(KK)
    DREL_t = np.ascontiguousarray(
        DREL.reshape(NCORES, NCH, P).transpose(0, 2, 1)
    ).astype(ml_dtypes.bfloat16)                       # [C, P, NCH]
    DRELB = np.ascontiguousarray(np.broadcast_to(
        DREL.astype(ml_dtypes.bfloat16).reshape(NCORES, 1, EPAD),
        (NCORES, P, EPAD)).transpose(1, 0, 2)).transpose(1, 0, 2)
    # wrapped int16 indices, per chunk 8 cols: [P, NCH*8]
    IDXW = np.zeros((NCORES, P, NCH * 8), dtype=np.int16)
    for c in range(NCORES):
        iv = SROWK[c].reshape(-1, 16)                  # [NCH*8, 16]
        w = iv.reshape(NCH, 8, 16).transpose(0, 2, 1).astype(np.int16)
        # per chunk: [16, 8] tiled to [128, 8]
        for ch in range(NCH):
            IDXW[c, :, ch * 8:(ch + 1) * 8] = np.tile(w[ch], (8, 1))
    L = dict(NCH=NCH, GW=GW, meta=meta, Ktot=[int(k) for k in Ktot])

    # per-core tile-ownership mask for alpha_dst slot accumulation
    MASKT = np.zeros((NCORES, P, NT), dtype=np.float32)
    for c in range(NCORES):
        t0 = c * NBLK
        t1 = min(NT, (c + 1) * NBLK)
        MASKT[c, :, t0:t1] = 1.0

    xT = np.zeros((D, NROW1), dtype=ml_dtypes.bfloat16)
    xT[:, :N] = x.T.astype(ml_dtypes.bfloat16)
    A1 = np.zeros((F1, 6), dtype=np.float32)
    for h in range(H):
        A1[h * HID:(h + 1) * HID, h] = as1[h]
        A1[h * HID:(h + 1) * HID, 3 + h] = ad1[h]
    A2 = np.stack([as2[0], ad2[0]], axis=1).astype(np.float32)

    shared = {
        "xT": xT,
        "RHS1": np.ascontiguousarray(np.concatenate(
            [W1, W1 @ A1], axis=1)).astype(ml_dtypes.bfloat16),
        "W2": np.ascontiguousarray(W2),
        "W2T": np.ascontiguousarray(W2.T),
        "A2": A2,
        "B1": np.ascontiguousarray(np.broadcast_to(b1, (P, F1))),
        "B2": np.ascontiguousarray(np.broadcast_to(b2, (P, F2))),
        "IOTAROW": np.ascontiguousarray(np.broadcast_to(
            np.tile(np.arange(P, dtype=ml_dtypes.bfloat16), GWMAX),
            (P, GWMAX * P))),
        "IOTACOL": np.ascontiguousarray(np.broadcast_to(
            np.arange(P, dtype=ml_dtypes.bfloat16).reshape(P, 1),
            (P, GWMAX * P))),
    }
    percore = []
    for c in range(NCORES):
        percore.append({
            "DREL1": DREL_t[c], "DRELB1": DRELB[c], "IDXW1": IDXW[c],
            "MASKT": MASKT[c],
        })
    key = tuple(L["Ktot"])
    return key, L, shared, percore


def _ap_view(ap, extra_offset, free_dims):
    import concourse.bass as bass

    return bass.AP(
        tensor=ap.tensor, offset=ap.offset + extra_offset,
        ap=[list(ap.ap[0])] + [list(d) for d in free_dims],
    )


def _dram_ap(dt_handle, offset, dims):
    import concourse.bass as bass

    ap = dt_handle.ap()
    return bass.AP(tensor=ap.tensor, offset=offset,
                   ap=[list(d) for d in dims])


def _build(L):
    import os
    global _SIM_Q0
    _SIM_Q0 = bool(int(os.environ.get("BASS_SIM_Q0", "0")))
    import concourse.bass as bass
    import concourse.bacc as bacc
    import concourse.tile as tile
    from concourse import mybir
    from concourse.masks import make_identity
    from concourse.library_config import mlp
    from contextlib import ExitStack

    f32 = mybir.dt.float32
    bf16 = mybir.dt.bfloat16
    i16 = mybir.dt.int16
    AT = mybir.ActivationFunctionType
    OP = mybir.AluOpType

    nc = bacc.Bacc("TRN2", target_bir_lowering=False, debug=False,
                   num_devices=NCORES, num_swdge_queues=4)

    NCH = L["NCH"]
    GW = L["GW"]
    assert GW <= GWMAX

    xT = nc.dram_tensor("xT", [D, NROW1], bf16, kind="ExternalInput")
    RHS1 = nc.dram_tensor("RHS1", [D, 198], bf16, kind="ExternalInput")
    W2 = nc.dram_tensor("W2", [F1, F2], f32, kind="ExternalInput")
    W2T = nc.dram_tensor("W2T", [F2, F1], f32, kind="ExternalInput")
    A2 = nc.dram_tensor("A2", [F2, 2], f32, kind="ExternalInput")
    B1 = nc.dram_tensor("B1", [P, F1], f32, kind="ExternalInput")
    B2 = nc.dram_tensor("B2", [P, F2], f32, kind="ExternalInput")
    IOTAROW = nc.dram_tensor("IOTAROW", [P, GWMAX * P], bf16,
                             kind="ExternalInput")
    IOTACOL = nc.dram_tensor("IOTACOL", [P, GWMAX * P], bf16,
                             kind="ExternalInput")
    DREL1 = nc.dram_tensor("DREL1", [P, NCH], bf16, kind="ExternalInput")
    DRELB1 = nc.dram_tensor("DRELB1", [P, NCH * P], bf16, kind="ExternalInput")
    IDXW1 = nc.dram_tensor("IDXW1", [P, NCH * 8], i16, kind="ExternalInput")
    MASKT = nc.dram_tensor("MASKT", [P, NT], f32, kind="ExternalInput")
    OUT = nc.dram_tensor("out", [NPC, F2], f32, kind="ExternalOutput")

    G1A = nc.dram_tensor("G1A", [NRA, G1W], bf16, kind="Internal")
    G1B = nc.dram_tensor("G1B", [NRB, G1W], bf16, kind="Internal")
    G2LA = nc.dram_tensor("G2LA", [RA, G2W], bf16, kind="Internal")
    G2LB = nc.dram_tensor("G2LB", [RB, G2W], bf16, kind="Internal")
    G2FA = nc.dram_tensor("G2FA", [NRA, G2W], bf16,
                          addr_space="Shared", kind="Internal")
    G2FB = nc.dram_tensor("G2FB", [NRB, G2W], bf16,
                          addr_space="Shared", kind="Internal")

    with tile.TileContext(nc) as tc, ExitStack() as ctx:
        consts = ctx.enter_context(tc.tile_pool(name="consts", bufs=1))
        sbA = ctx.enter_context(tc.tile_pool(name="sbA", bufs=4))
        pas = ctx.enter_context(tc.tile_pool(name="pas", bufs=3, space="PSUM"))
        pse = ctx.enter_context(tc.tile_pool(name="pse", bufs=2, space="PSUM"))
        psad = ctx.enter_context(tc.tile_pool(name="psad", bufs=1, space="PSUM"))
        pst = ctx.enter_context(tc.tile_pool(name="pst", bufs=1, space="PSUM"))
        pge = ctx.enter_context(tc.tile_pool(name="pge", bufs=1, space="PSUM"))
        gpool = ctx.enter_context(tc.tile_pool(name="gpool", bufs=6))
        fpool = ctx.enter_context(tc.tile_pool(name="fpool", bufs=4))
        spool = ctx.enter_context(tc.tile_pool(name="spool", bufs=5))
        ipool = ctx.enter_context(tc.tile_pool(name="ipool", bufs=6))
        epool = ctx.enter_context(tc.tile_pool(name="epool", bufs=4))

        nc.gpsimd.load_library(mlp)

        # ---------------- constants / preloads ----------------
        iotarow = consts.tile([P, GWMAX * P], bf16)
        nc.sync.dma_start(out=iotarow[:], in_=IOTAROW[:])
        iotacol = consts.tile([P, GWMAX * P], bf16)
        nc.sync.dma_start(out=iotacol[:], in_=IOTACOL[:])
        b1t = consts.tile([P, F1], f32)
        nc.sync.dma_start(out=b1t[:], in_=B1[:])
        b2t = consts.tile([P, F2], f32)
        nc.sync.dma_start(out=b2t[:], in_=B2[:])
        ident = consts.tile([P, P], f32)
        make_identity(nc, ident[:])

        idxall = consts.tile([P, NCH * 8], i16)
        nc.sync.dma_start(out=idxall[:], in_=IDXW1[:])
        drelall = consts.tile([P, NCH], bf16)
        nc.sync.dma_start(out=drelall[:], in_=DREL1[:])
        maskt = consts.tile([P, NT], f32)
        nc.sync.dma_start(out=maskt[:], in_=MASKT[:])

        rhs1 = consts.tile([P, 198], bf16)
        nc.sync.dma_start(out=rhs1[:], in_=RHS1[:])

        w2t = consts.tile([F2, F1], f32)
        nc.sync.dma_start(out=w2t[:], in_=W2T[:])
        a2t = consts.tile([F2, 2], f32)
        nc.sync.dma_start(out=a2t[:], in_=A2[:])
        rhs2_lo = consts.tile([P, 66], f32)
        nc.sync.dma_start(out=rhs2_lo[:, :F2], in_=W2[0:P, :])
        rhs2_hi = consts.tile([F1 - P, 66], f32)
        nc.sync.dma_start(out=rhs2_hi[:, :F2], in_=W2[P:F1, :])
        pu2 = pst.tile([P, 256], f32, tag="tr")
        nc.tensor.matmul(out=pu2[:, :2], lhsT=w2t[:, 0:P], rhs=a2t[:],
                         start=True, stop=True)
        nc.vector.tensor_copy(out=rhs2_lo[:, F2:F2 + 2], in_=pu2[:, :2])
        pu2b = pst.tile([P, 256], f32, tag="tr")
        nc.tensor.matmul(out=pu2b[:F1 - P, 4:6], lhsT=w2t[:, P:F1], rhs=a2t[:],
                         start=True, stop=True)
        nc.vector.tensor_copy(out=rhs2_hi[:, F2:F2 + 2], in_=pu2b[:F1 - P, 4:6])

        # alpha_dst-block tables (slot-major, partition = node-within-slot)
        adsb = consts.tile([P, NBLK, 4], f32)
        nc.vector.memset(adsb[:], 0.0)
        adbh1 = consts.tile([P, NBLK, 4], bf16)
        adb2_sb = consts.tile([P, NBLK, 1], bf16)

        # ---------------- stage A (piece-A tiles first), batched -------------
        def stage_a_group(t0, nb):
            c = t0 // NBLK
            i0 = t0 - c * NBLK
            xt = sbA.tile([P, BA * P], bf16, tag="xt")
            nc.sync.dma_start(out=xt[:, :nb * P],
                              in_=xT[:, t0 * P:(t0 + nb) * P])
            gbf = sbA.tile([P, BA, G1W], bf16, tag="gbf")
            gf32 = gbf[:].bitcast(f32)
            nc.vector.memset(gf32[:, :, 99:128], 0.0)
            for b in range(nb):
                t = t0 + b
                pa = pas.tile([P, 200], f32, tag="pa")
                nc.tensor.matmul(out=pa[:, :198], lhsT=xt[:, b * P:(b + 1) * P],
                                 rhs=rhs1[:], start=True, stop=True)
                if b % 2 == 0:
                    nc.scalar.activation(out=gbf[:, b, :F1], in_=pa[:, :F1],
                                         func=AT.Copy)
                else:
                    nc.vector.tensor_copy(out=gbf[:, b, :F1], in_=pa[:, :F1])
                nc.vector.tensor_copy(out=gf32[:, b, 96:99],
                                      in_=pa[:, F1:F1 + 3])
                adm = sbA.tile([P, 4], f32, tag="adm")
                nc.vector.tensor_scalar(
                    out=adm[:, :3], in0=pa[:, F1 + 3:F1 + 6],
                    scalar1=maskt[:, t:t + 1], scalar2=None, op0=OP.mult)
                col = t % NBLK
                nc.vector.tensor_tensor(
                    out=adsb[:, col, :3], in0=adsb[:, col, :3],
                    in1=adm[:, :3], op=OP.add)
            if i0 < PA_T:
                dst_ap = _dram_ap(G1A, (c * RA + i0 * P) * G1W,
                                  [[G1W, P], [P * G1W, nb], [1, G1W]])
            else:
                dst_ap = _dram_ap(G1B, (c * RB + (i0 - PA_T) * P) * G1W,
                                  [[G1W, P], [P * G1W, nb], [1, G1W]])
            nc.scalar.dma_start(out=dst_ap, in_=gbf[:, :nb, :])

        # piece-A tiles (all cores) first, then piece-B
        for piece in (0, 1):
            for c in range(NCORES):
                lo = c * NBLK + (0 if piece == 0 else PA_T)
                hi = min(c * NBLK + (PA_T if piece == 0 else NBLK),
                         NT)
                t = lo
                while t < hi:
                    nb = min(BA, hi - t)
                    stage_a_group(t, nb)
                    t += nb

        nc.vector.tensor_copy(out=adbh1[:], in_=adsb[:])

        # ---------------- generic edge phase (one slot per iteration) -------
        gq = [0]

        def edge_layer(TBLa, TBLb, width, nfeat, as_f32col, adbh, nheads,
                       ps_width, slot_epilogue, after_slot=None):
            meta = L["meta"]
            fw = nfeat + nheads
            hd = nfeat // nheads
            for s in range(NBLK):
                c0, ka, kb = meta[s]
                nch = ka + kb
                drelb_t = ipool.tile([P, GW * P], bf16, tag="drelb",
                                     name="drelb_t")
                nc.sync.dma_start(out=drelb_t[:, :nch * P],
                                  in_=DRELB1[:, c0 * P:(c0 + nch) * P])
                grow = gpool.tile([P, GW, width], bf16, tag="grow", name="grow")
                for tbl, lo, cnt in ((TBLa, 0, ka), (TBLb, ka, kb)):
                    j = 0
                    while j < cnt:
                        nj = min(8, cnt - j)
                        cj = c0 + lo + j
                        nc.gpsimd.dma_gather(
                            grow[:, lo + j:lo + j + nj, :], tbl[:],
                            idxall[:, cj * 8:(cj + nj) * 8], nj * P, nj * P,
                            width, queue_num=0 if _SIM_Q0 else gq[0] % 4)
                        gq[0] += 1
                        j += nj
                # S8: [e_part, chunk, d] one-hot (bf16)
                S8 = spool.tile([P, GW * P], bf16, tag="s8", name="s8")
                nc.vector.tensor_tensor(
                    out=_ap_view(S8[:], 0, [[P, nch], [1, P]]),
                    in0=_ap_view(drelall[:], c0, [[1, nch], [0, P]]),
                    in1=_ap_view(iotarow[:], 0, [[P, nch], [1, P]]),
                    op=OP.is_equal)
                # S_T: [d_part, chunk*P + e] one-hot; all-bf16 packed (2x DVE)
                st8 = spool.tile([P, GW * P], bf16, tag="st8", name="st8")
                nc.vector.tensor_tensor(
                    out=st8[:, :nch * P], in0=drelb_t[:, :nch * P],
                    in1=iotacol[:, :nch * P], op=OP.is_equal)
                # alpha_dst expansion: adp[e, ch*nheads+h]
                ps = pse.tile([P, 200], f32, tag="ps", name="ps")
                adp = psad.tile([P, GWMAX * H], f32, tag="adp", name="adp")
                for jj in range(nch):
                    nc.tensor.matmul(
                        out=adp[:, jj * nheads:(jj + 1) * nheads],
                        lhsT=st8[:, jj * P:(jj + 1) * P],
                        rhs=adbh[:, s, :nheads],
                        start=True, stop=True)
                # logits -> exp -> weighted features
                growf = grow[:].bitcast(f32)
                t8 = epool.tile([P, GWMAX * H], f32, tag="t8", name="t8")
                nc.vector.tensor_tensor(
                    out=_ap_view(t8[:], 0, [[nheads, nch], [1, nheads]]),
                    in0=_ap_view(growf, as_f32col,
                                 [[width // 2, nch], [1, nheads]]),
                    in1=_ap_view(adp[:], 0, [[nheads, nch], [1, nheads]]),
                    op=OP.add)
                # exp(lrelu(t)) == max(exp(t), exp(SLOPE*t)) exactly
                e2 = epool.tile([P, GWMAX * H], f32, tag="e2", name="e2")
                nc.scalar.activation(out=e2[:, :nch * nheads],
                                     in_=t8[:, :nch * nheads],
                                     func=AT.Exp, scale=SLOPE)
                F8 = fpool.tile([P, GW * fw], bf16, tag="f8", name="f8")
                nc.scalar.activation(
                    out=_ap_view(F8[:], nfeat, [[fw, nch], [1, nheads]]),
                    in_=_ap_view(t8[:], 0, [[nheads, nch], [1, nheads]]),
                    func=AT.Exp)
                nc.vector.tensor_tensor(
                    out=_ap_view(F8[:], nfeat, [[fw, nch], [1, nheads]]),
                    in0=_ap_view(F8[:], nfeat, [[fw, nch], [1, nheads]]),
                    in1=_ap_view(e2[:], 0, [[nheads, nch], [1, nheads]]),
                    op=OP.max)
                nc.vector.tensor_tensor(
                    out=_ap_view(F8[:], 0, [[fw, nch], [hd, nheads], [1, hd]]),
                    in0=_ap_view(grow[:], 0,
                                 [[width, nch], [hd, nheads], [1, hd]]),
                    in1=_ap_view(F8[:], nfeat,
                                 [[fw, nch], [1, nheads], [0, hd]]),
                    op=OP.mult)
                for jj in range(nch):
                    nc.tensor.matmul(
                        out=ps[:, :ps_width],
                        lhsT=S8[:, jj * P:(jj + 1) * P],
                        rhs=F8[:, jj * fw:jj * fw + ps_width],
                        start=(jj == 0), stop=(jj == nch - 1))
                slot_epilogue(s, ps)
                if after_slot is not None:
                    after_slot(s)

        # L1 epilogue: h -> transpose -> G2 rows + ad2 (SBUF)
        def epi1(s, ps):
            rc = epool.tile([P, H], f32, tag="rc", name="rc")
            nc.vector.tensor_scalar_add(out=rc[:], in0=ps[:, F1:F1 + H],
                                        scalar1=EPS)
            rc2 = epool.tile([P, H], f32, tag="rc2", name="rc2")
            nc.vector.reciprocal(out=rc2[:], in_=rc[:])
            hm = epool.tile([P, F1], f32, tag="hm", name="hm")
            nc.vector.tensor_tensor(
                out=_ap_view(hm[:], 0, [[HID, H], [1, HID]]),
                in0=_ap_view(ps[:, :F1], 0, [[HID, H], [1, HID]]),
                in1=_ap_view(rc2[:], 0, [[1, H], [0, HID]]),
                op=OP.mult)
            hb = epool.tile([P, F1], f32, tag="hb", name="hb")
            nc.vector.tensor_tensor(out=hb[:], in0=hm[:], in1=b1t[:], op=OP.add)
            hr = epool.tile([P, F1], f32, tag="hr", name="hr")
            nc.scalar.activation(out=hr[:], in_=hb[:], func=AT.Relu)
            pt = pst.tile([P, 256], f32, tag="tr", name="pt")
            nc.tensor.transpose(out=pt[:, 0:P], in_=hr[:, :P], identity=ident[:])
            nc.tensor.transpose(out=pt[:F1 - P, P:256], in_=hr[:, P:F1],
                                identity=ident[:])
            ht1 = epool.tile([P, P], f32, tag="ht1", name="ht1")
            nc.vector.tensor_copy(out=ht1[:], in_=pt[:, 0:P])
            ht2 = epool.tile([F1 - P, P], f32, tag="ht2", name="ht2")
            nc.vector.tensor_copy(out=ht2[:], in_=pt[:F1 - P, P:256])
            pg = pge.tile([P, 200], f32, tag="pg", name="pg")
            nc.tensor.matmul(out=pg[:, :66], lhsT=ht1[:], rhs=rhs2_lo[:],
                             start=True, stop=False)
            nc.tensor.matmul(out=pg[:, :66], lhsT=ht2[:], rhs=rhs2_hi[:],
                             start=False, stop=True)
            g2 = epool.tile([P, G2W], bf16, tag="g2", name="g2")
            nc.vector.tensor_copy(out=g2[:, :F2], in_=pg[:, :F2])
            g2f = g2[:].bitcast(f32)
            nc.vector.memset(g2f[:, 33:64], 0.0)
            nc.vector.tensor_copy(out=g2f[:, 32:33], in_=pg[:, F2:F2 + 1])
            nc.vector.tensor_copy(out=adb2_sb[:, s, :1], in_=pg[:, F2 + 1:F2 + 2])
            if s < PA_T:
                nc.sync.dma_start(out=G2LA[s * P:(s + 1) * P, :], in_=g2[:])
            else:
                sb = s - PA_T
                nc.sync.dma_start(out=G2LB[sb * P:(sb + 1) * P, :], in_=g2[:])

        def after1(s):
            from concourse import mybir as mb
            if s == PA_T - 1:
                nc.gpsimd.collective_compute(
                    "AllGather", mb.AluOpType.bypass,
                    replica_groups=[list(range(NCORES))],
                    ins=[G2LA.ap().opt()], outs=[G2FA.ap().opt()])
            if s == NBLK - 1:
                nc.gpsimd.collective_compute(
                    "AllGather", mb.AluOpType.bypass,
                    replica_groups=[list(range(NCORES))],
                    ins=[G2LB.ap().opt()], outs=[G2FB.ap().opt()])

        edge_layer(G1A, G1B, G1W, F1, 96, adbh1, H, F1 + H, epi1,
                   after_slot=after1)

        # ---------------- layer 2 ----------------
        def epi2(s, ps):
            rc = epool.tile([P, 1], f32, tag="rcB", name="rcB")
            nc.vector.tensor_scalar_add(out=rc[:], in0=ps[:, F2:F2 + 1],
                                        scalar1=EPS)
            rc2 = epool.tile([P, 1], f32, tag="rcB2", name="rcB2")
            nc.vector.reciprocal(out=rc2[:], in_=rc[:])
            om = epool.tile([P, F2], f32, tag="om", name="om")
            nc.vector.tensor_tensor(out=om[:], in0=ps[:, :F2],
                                    in1=rc2[:].to_broadcast([P, F2]),
                                    op=OP.mult)
            ob = epool.tile([P, F2], f32, tag="ob", name="ob")
            nc.vector.tensor_tensor(out=ob[:], in0=om[:], in1=b2t[:], op=OP.add)
            orl = epool.tile([P, F2], f32, tag="orl", name="orl")
            nc.scalar.activation(out=orl[:], in_=ob[:], func=AT.Relu)
            nc.sync.dma_start(out=OUT[s * P:(s + 1) * P, :], in_=orl[:])

        edge_layer(G2FA, G2FB, G2W, F2, 32, adb2_sb, 1, F2 + 1, epi2)

    nc.compile()
    return nc


def _get_compiled(key, L):
    if key not in _compiled:
        _compiled[key] = _build(L)
    return _compiled[key]


def run(inputs, **runkw):
    from concourse import bass_utils

    key, L, shared, percore = _host_prep(inputs)
    nc = _get_compiled(key, L)
    in_maps = []
    for c in range(NCORES):
        m = dict(shared)
        m.update(percore[c])
        in_maps.append(m)
    res = bass_utils.run_bass_kernel_spmd(
        nc, in_maps, core_ids=list(range(NCORES)), **runkw)
    return res


def assemble(results):
    out = np.empty((N, F2), dtype=np.float32)
    for c in range(NCORES):
        lo = c * NPC
        valid = min(NPC, N - lo)
        out[lo:lo + valid] = results[c]["out"][:valid]
    return out


def kernel(**inputs):
    res = run(inputs)
    return assemble(res.results)
